# revision 14
# baseline (speedup 1.0000x reference)
"""BLOOM-style attention block (QKV proj + ALiBi causal attention + dense + residual)
for Trainium2, SPMD over 8 NeuronCores.

Sharding: core c -> (b = c // 4, head group g = c % 4, heads [4g..4g+4)).

v2: fp8 (e4m3) DoubleRow matmuls for the QKV projections and probs@V;
bf16 for scores and dense (error budget: ~1.6e-2 vs 2e-2 gate).

Per core (b, g):
  - hT packed as K=256 DoubleRow pairs [128, 8, 2, S] fp8 (x32 scale)
  - q/k proj DR (Wqk fp8 x1024): 8 DR matmuls per [128,512] tile; ACT applies
    scale (alpha/32768 for q, 1/32768 for k) + bias -> bf16 tiles
  - v proj DR -> +bias via ones(16)xfp8(bv*2048) matmul -> ACT x(1/1024) -> fp8 (32*v)
    stored as k-tile pairs v2 [128, 2, DQ]
  - scores bf16 transposed per (head, q-block): s_T [k128, q512]
  - additive -30000 mask (DVE) on partial tiles; exp via ACT with per-partition
    alibi+ln2 bias -> fp8 probs (2*p) written into pair tiles pt2 [128, 2, 512]
  - ctx DR (v2 x pt2) accumulated in PSUM; denominator via ones-DR matmul into
    a [1,512] PSUM accumulator (no DVE/gpsimd reduction)
  - rec = 1/d (DVE), broadcast via gpsimd, ctx normalized by DVE mul -> bf16
  - dense bf16 per q-block interleaved after each attn block; ACT copy x(1/32)
Host: out[b] = sum_g partial[b,g] + bd + residual[b].
"""

import numpy as np
import ml_dtypes

import concourse.bacc as bacc
import concourse.mybir as mybir
from concourse import tile
from concourse.bass_utils import run_bass_kernel_spmd

B, S, H, NH, HD = 2, 2048, 2048, 16, 128
ALPHA = 1.0 / float(np.sqrt(HD))
P = 128
NCORES = 8
HPC = 4            # heads per core
DQ = HPC * HD      # 512 = per-core q/k/v width
NJ = S // 512      # 4 q blocks of 512
NKT = S // P       # 16 k tiles of 128
NKH2 = H // 256    # 8 DoubleRow contraction pairs for projections
NCOL = 2 * DQ // P  # 8 qk col tiles (c<4: q head c, c>=4: k head c-4)
F32 = mybir.dt.float32
BF16 = mybir.dt.bfloat16
F8 = mybir.dt.float8e4
BF = ml_dtypes.bfloat16
E4 = ml_dtypes.float8_e4m3fn

SH = 32.0        # hidden fp8 scale
SW = 1024.0      # weight fp8 scale
SV = 32.0        # v fp8 scale (= SH*SW / 1024)
SP = 1.0         # prob fp8 scale (folded into alibi bias; 1.0: max unnorm prob ~167 << 448)
LN_SP = float(np.log(SP))

_cache = {}


def _analyze_mask(mask_b):
    """mask_b: [S, S] bool, True == masked out. Per (i, J) k/q tile:
    'skip' | pattern-index | None. Patterns are additive [k128, q512] f32."""
    patterns, pat_index, tilemap = [], {}, {}
    for J in range(NJ):
        for i in range(NKT):
            sub = mask_b[512 * J:512 * J + 512, P * i:P * i + P]  # [q, k]
            if sub.all():
                tilemap[(i, J)] = 'skip'
            elif not sub.any():
                tilemap[(i, J)] = None
            else:
                pat = np.where(sub.T, np.float32(-30000.0), np.float32(0.0))
                key = pat.tobytes()
                if key not in pat_index:
                    pat_index[key] = len(patterns)
                    patterns.append(pat)
                tilemap[(i, J)] = pat_index[key]
    return tilemap, patterns


def _build_program(tilemap, npat):
    nc = bacc.Bacc(None, target_bir_lowering=False, debug=False)

    hT = nc.dram_tensor("hT", [P, NKH2, 2, S], F8, kind="ExternalInput")
    Wqk = nc.dram_tensor("Wqk", [P, NKH2, 2, 2 * DQ], F8, kind="ExternalInput")
    Wv = nc.dram_tensor("Wv", [P, NKH2, 2, DQ], F8, kind="ExternalInput")
    bqk = nc.dram_tensor("bqk", [P, NCOL], F32, kind="ExternalInput")
    bv = nc.dram_tensor("bv", [1, DQ], F8, kind="ExternalInput")
    alib = nc.dram_tensor("alib", [P, HPC * NKT], F32, kind="ExternalInput")
    Wd = nc.dram_tensor("Wd", [P, HPC, H], BF16, kind="ExternalInput")
    masks = nc.dram_tensor("masks", [P, max(npat, 1) * 512], F32, kind="ExternalInput")
    out = nc.dram_tensor("out", [S, H], BF16, kind="ExternalOutput")

    with tile.TileContext(nc) as tc:
        with (
            tc.tile_pool(name="wqk", bufs=1) as wqk_pool,
            tc.tile_pool(name="wv", bufs=1) as wv_pool,
            tc.tile_pool(name="wd", bufs=1) as wd_pool,
            tc.tile_pool(name="consts", bufs=1) as consts,
            tc.tile_pool(name="hts", bufs=NJ) as ht_pool,
            tc.tile_pool(name="qkt", bufs=NCOL * NJ + 1) as qkt_pool,
            tc.tile_pool(name="vsb", bufs=NKT // 2 + 1) as v_pool,
            tc.tile_pool(name="prob", bufs=5) as prob_pool,
            tc.tile_pool(name="dstat", bufs=7) as dstat_pool,
            tc.tile_pool(name="ctxt", bufs=HPC * NJ + 1) as ctxt_pool,
            tc.tile_pool(name="ostage", bufs=3) as ostage_pool,
            tc.tile_pool(name="pp_proj", bufs=2, space="PSUM") as pp_proj,
            tc.tile_pool(name="pp_score", bufs=2, space="PSUM") as pp_score,
            tc.tile_pool(name="pp_ctx", bufs=2, space="PSUM") as pp_ctx,
            tc.tile_pool(name="pp_d", bufs=2, space="PSUM") as pp_d,
        ):
            # ---- constants ----
            mask_sb = consts.tile([P, max(npat, 1) * 512], F32)
            nc.sync.dma_start(out=mask_sb[:], in_=masks[:])
            al_sb = consts.tile([P, HPC * NKT], F32)
            nc.sync.dma_start(out=al_sb[:], in_=alib[:])
            bqk_sb = consts.tile([P, NCOL], F32)
            nc.sync.dma_start(out=bqk_sb[:], in_=bqk[:])
            bv_sb = consts.tile([1, DQ], F8)
            nc.sync.dma_start(out=bv_sb[:], in_=bv[:])
            ones16 = consts.tile([1, P], F8)
            nc.any.memset(ones16[:], 16.0)
            ones8 = consts.tile([P, 2, 32], F8)
            nc.any.memset(ones8[:], 1.0)

            # ---- batched input DMAs (issue cost ~0.66us each on the Sync
            # queue dominates with many small transfers): one DMA per hT
            # block, two for Wqk (lo/hi cols), one each for Wv / Wd ----
            ht_sb = {}  # (j, kh2) -> [128, 2, 512] fp8 view

            def load_ht(j):
                t = ht_pool.tile([P, NKH2, 2, 512], F8, tag="ht", name=f"htb{j}")
                nc.sync.dma_start(out=t[:], in_=hT[:, :, :, 512 * j:512 * j + 512])
                for kh2 in range(NKH2):
                    ht_sb[(j, kh2)] = t[:, kh2, :, :]

            load_ht(0)
            wqk_big = wqk_pool.tile([P, NKH2, 2, 2 * DQ], F8, tag="wqk")
            nc.sync.dma_start(out=wqk_big[:, :, :, 0:DQ], in_=Wqk[:, :, :, 0:DQ])
            nc.sync.dma_start(
                out=wqk_big[:, :, :, DQ:2 * DQ], in_=Wqk[:, :, :, DQ:2 * DQ])
            wqk_sb = [wqk_big[:, kh2, :, :] for kh2 in range(NKH2)]
            wv_big = wv_pool.tile([P, NKH2, 2, DQ], F8, tag="wv")
            nc.sync.dma_start(out=wv_big[:], in_=Wv[:])
            wv_sb = [wv_big[:, kh2, :, :] for kh2 in range(NKH2)]
            load_ht(1)
            load_ht(2)
            load_ht(3)
            wd_big = wd_pool.tile([P, HPC, H], BF16, tag="wd")
            nc.sync.dma_start(out=wd_big[:], in_=Wd[:])
            wd_sb = {}  # (kd, cb) -> [128, 512] bf16 view
            for kd in range(HPC):
                for cb in range(NJ):
                    wd_sb[(kd, cb)] = wd_big[:, kd, 512 * cb:512 * cb + 512]

            qkt_sb = {}   # (c, j) -> [128, 512] bf16; c<4: q head c (alpha-scaled), c>=4: k
            v_sb = {}     # pr -> [128, 2, DQ] fp8 (32*v), k-tiles (2pr, 2pr+1)
            ctxt_sb = {}  # (h, J) -> [128, 512] bf16 (32*ctx)

            def proj_sweeps(j):
                sweeps = []

                def qk_sweep(cg, j=j):
                    ps = [pp_proj.tile([P, 512], F32, tag="ps", name=f"ps{j}_{cg}_{_i}")
                          for _i in range(2)]
                    for kh2 in range(NKH2):
                        for cc in range(2):
                            c = 2 * cg + cc
                            nc.tensor.matmul(
                                ps[cc][:],
                                wqk_sb[kh2][:, :, P * c:P * c + P],
                                ht_sb[(j, kh2)][:],
                                start=(kh2 == 0), stop=(kh2 == NKH2 - 1),
                                perf_mode=mybir.MatmulPerfMode.DoubleRow,
                            )
                    for cc in range(2):
                        c = 2 * cg + cc
                        sc = (ALPHA / (SH * SW)) if c < HPC else (1.0 / (SH * SW))
                        qt = qkt_pool.tile([P, 512], BF16, tag="qkt")
                        nc.vector.tensor_scalar(
                            qt[:], ps[cc][:], sc, bqk_sb[:, c:c + 1],
                            mybir.AluOpType.mult, mybir.AluOpType.add)
                        qkt_sb[(c, j)] = qt

                def v_sweep(vg, j=j):
                    pv = [pp_proj.tile([P, DQ], F32, tag="ps", name=f"pv{j}_{vg}_{_i}")
                          for _i in range(2)]
                    for kh2 in range(NKH2):
                        for mm in range(2):
                            m = 2 * vg + mm
                            nc.tensor.matmul(
                                pv[mm][:],
                                ht_sb[(j, kh2)][:, :, P * m:P * m + P],
                                wv_sb[kh2][:],
                                start=(kh2 == 0), stop=False,
                                perf_mode=mybir.MatmulPerfMode.DoubleRow,
                            )
                    for mm in range(2):
                        m = 4 * j + 2 * vg + mm
                        nc.tensor.matmul(
                            pv[mm][:], ones16[:], bv_sb[:], start=False, stop=True)
                        if m % 2 == 0:
                            vt = v_pool.tile([P, 2, DQ], F8, tag="v",
                                             name=f"v{m // 2}")
                            v_sb[m // 2] = vt
                        nc.scalar.activation(
                            v_sb[m // 2][:, m % 2, :], pv[mm][:],
                            mybir.ActivationFunctionType.Identity,
                            scale=1.0 / SW)

                for cg in range(4):
                    sweeps.append(lambda cg=cg: qk_sweep(cg))
                for vg in range(2):
                    sweeps.append(lambda vg=vg: v_sweep(vg))
                return sweeps

            def attn_head(h, J, pctx, pd):
                """Emit one pair-step generator for head h, block J."""
                needed = [i for i in range(NKT) if tilemap[(i, J)] != 'skip']
                npair = (len(needed) + 1) // 2
                for pr in range(npair):
                    pt2 = prob_pool.tile([P, 2, 512], F8, tag="pt")
                    halves = needed[2 * pr:2 * pr + 2]
                    for half, i in enumerate(halves):
                        pscore = pp_score.tile([P, 512], F32, tag="pscore")
                        nc.tensor.matmul(
                            pscore[:],
                            qkt_sb[(HPC + h, i // 4)][:, P * (i % 4):P * (i % 4) + P],
                            qkt_sb[(h, J)][:],
                            start=True, stop=True,
                        )
                        pat = tilemap[(i, J)]
                        if pat is not None:
                            nc.vector.tensor_add(
                                pscore[:], pscore[:],
                                mask_sb[:, 512 * pat:512 * pat + 512])
                        nc.scalar.activation(
                            pt2[:, half, :], pscore[:],
                            mybir.ActivationFunctionType.Exp,
                            bias=al_sb[:, h * NKT + i:h * NKT + i + 1],
                        )
                    if len(halves) == 1:
                        nc.any.memset(pt2[:, 1, :], 0.0)
                    yield
                    nc.tensor.matmul(
                        pctx[:],
                        v_sb[needed[2 * pr] // 2][:, :, P * h:P * h + P],
                        pt2[:],
                        start=(pr == 0), stop=(pr == npair - 1),
                        perf_mode=mybir.MatmulPerfMode.DoubleRow,
                    )
                    nc.tensor.matmul(
                        pd[:], ones8[:], pt2[:],
                        start=(pr == 0), stop=(pr == npair - 1),
                        perf_mode=mybir.MatmulPerfMode.DoubleRow,
                    )
                    yield

            def finish_head(h, J, pctx, pd):
                deps = dstat_pool.tile([1, 512], F32, tag="deps")
                nc.vector.tensor_scalar_add(deps[:], pd[0:1, :], 1e-12)
                rec = dstat_pool.tile([1, 512], F32, tag="rec")
                nc.vector.reciprocal_approx_fast(rec[:], deps[:])
                recb = dstat_pool.tile([P, 512], F32, tag="recb")
                nc.gpsimd.partition_broadcast(recb[:], rec[:], 128)
                ct = ctxt_pool.tile([P, 512], BF16, tag="ctxt")
                nc.vector.tensor_mul(ct[:], pctx[:], recb[:])
                ctxt_sb[(h, J)] = ct

            def attn_block(J):
                """Generator: yields after each interleaved 2-head round."""
                for hp in range(2):
                    h0, h1 = 2 * hp, 2 * hp + 1
                    pctx0 = pp_ctx.tile([P, 512], F32, tag="pctx", name=f"pc{J}_{h0}")
                    pctx1 = pp_ctx.tile([P, 512], F32, tag="pctx", name=f"pc{J}_{h1}")
                    pd0 = pp_d.tile([32, 512], F32, tag="pd", name=f"pd{J}_{h0}")
                    pd1 = pp_d.tile([32, 512], F32, tag="pd", name=f"pd{J}_{h1}")
                    gens = [(attn_head(h0, J, pctx0, pd0), h0, pctx0, pd0),
                            (attn_head(h1, J, pctx1, pd1), h1, pctx1, pd1)]
                    live = list(gens)
                    while live:
                        for item in list(live):
                            g, h, pc, pdd = item
                            try:
                                next(g)
                            except StopIteration:
                                finish_head(h, J, pc, pdd)
                                live.remove(item)
                        yield

            def dense_groups(J, pools=None):
                groups = []
                pools = pools or [pp_proj]
                obig = {}

                def grp(cb, mm, J=J):
                    pool = pools[(4 * mm + cb) % len(pools)]
                    tg = "ps" if pool is pp_proj else "pscore"
                    m = 4 * J + mm
                    pdn = pool.tile([P, 512], F32, tag=tg, name=f"dn{J}_{cb}_{mm}")
                    for kd in range(HPC):
                        nc.tensor.matmul(
                            pdn[:],
                            ctxt_sb[(kd, J)][:, P * mm:P * mm + P],
                            wd_sb[(kd, cb)][:],
                            start=(kd == 0), stop=(kd == HPC - 1),
                        )
                    if mm not in obig:
                        obig[mm] = ostage_pool.tile(
                            [P, H], BF16, tag="ot", name=f"ob{J}_{mm}")
                    nc.vector.tensor_scalar_mul(
                        obig[mm][:, 512 * cb:512 * cb + 512], pdn[:], 1.0 / SV)
                    if cb == NJ - 1:
                        nc.sync.dma_start(
                            out=out[P * m:P * m + P, :], in_=obig[mm][:])

                for mm in range(4):
                    for cb in range(NJ):
                        groups.append(lambda cb=cb, mm=mm: grp(cb, mm))
                return groups

            # ---- master schedule: proj(j) sweeps interleaved with
            # attn(j-1) rounds; dense groups pumped into attn(3) bubbles ----
            dense_q = []

            def pump(n):
                while n > 0 and dense_q:
                    dense_q.pop(0)()
                    n -= 1

            for sw in proj_sweeps(0):
                sw()
            for j in range(1, NJ):
                rounds = attn_block(j - 1)
                sweeps = proj_sweeps(j)
                npair_prev = 2 * (j - 1) + 2
                nround = 2 * (2 * npair_prev + 1)  # rounds incl. finish
                per = max(1, (nround + len(sweeps) - 1) // len(sweeps))
                exhausted = False
                for sw in sweeps:
                    sw()
                    for _ in range(per):
                        try:
                            next(rounds)
                        except StopIteration:
                            exhausted = True
                            break
                    pump(1)
                while not exhausted:
                    try:
                        next(rounds)
                        pump(1)
                    except StopIteration:
                        exhausted = True
                dense_q.extend(dense_groups(j - 1))
            for _ in attn_block(NJ - 1):
                pump(3)
            dense_q.extend(dense_groups(NJ - 1, pools=[pp_proj, pp_score]))
            pump(len(dense_q))

    nc.finalize()
    return nc


def _pack_dr(mat, scale):
    """[H, C] f32 -> [128, NKH2, 2, C] fp8 DoubleRow pairs, scaled."""
    h, c = mat.shape
    m = np.clip(mat * scale, -448.0, 448.0).astype(E4)
    return np.ascontiguousarray(m.reshape(NKH2, 2, P, c).transpose(2, 0, 1, 3))


def kernel(hidden_states, attention_mask, residual, alibi, Wqkv, bqkv, Wd, bd):
    hidden_states = np.asarray(hidden_states, np.float32)
    attention_mask = np.asarray(attention_mask).astype(bool)
    residual = np.asarray(residual, np.float32)
    alibi = np.asarray(alibi, np.float32)
    Wqkv = np.asarray(Wqkv, np.float32)
    bqkv = np.asarray(bqkv, np.float32)
    Wd = np.asarray(Wd, np.float32)
    bd = np.asarray(bd, np.float32)

    m0 = attention_mask[0, 0]
    for b in range(1, B):
        assert np.array_equal(attention_mask[b, 0], m0), "per-batch masks differ"
    tilemap, patterns = _analyze_mask(m0)
    npat = len(patterns)
    assert npat <= 8, f"too many mask patterns: {npat}"
    mask_host = np.ascontiguousarray(
        np.concatenate(patterns, axis=1) if npat else np.zeros((P, 512), np.float32))

    key = tuple(sorted((k, str(v)) for k, v in tilemap.items())) + (npat,)
    if key not in _cache:
        _cache[key] = _build_program(tilemap, npat)
    nc = _cache[key]

    Wq3 = Wqkv.reshape(H, NH, 3, HD)   # col = nh*384 + {0:k,1:q,2:v}*128 + d
    bq3 = bqkv.reshape(NH, 3, HD)

    hT_cores = {}
    for b in range(B):
        hT_cores[b] = _pack_dr(np.ascontiguousarray(hidden_states[b].T), SH)

    in_maps = []
    for core in range(NCORES):
        b, g = divmod(core, HPC)
        hs = [HPC * g + t for t in range(HPC)]
        Wq = np.concatenate([Wq3[:, h, 1, :] for h in hs], 1)
        Wk = np.concatenate([Wq3[:, h, 0, :] for h in hs], 1)
        Wv_ = np.concatenate([Wq3[:, h, 2, :] for h in hs], 1)
        bq = np.concatenate([bq3[h, 1, :] for h in hs]) * ALPHA
        bk = np.concatenate([bq3[h, 0, :] for h in hs])
        bv_ = np.concatenate([bq3[h, 2, :] for h in hs])
        bqk_h = np.concatenate([bq, bk]).reshape(NCOL, P).T
        al_h = np.stack([alibi[b * NH + h, 0] for h in hs], 0) + LN_SP  # [HPC, S]
        al_sb = al_h.reshape(HPC, NKT, P).transpose(2, 0, 1).reshape(P, HPC * NKT)
        in_maps.append({
            "hT": hT_cores[b],
            "Wqk": _pack_dr(np.concatenate([Wq, Wk], 1), SW),
            "Wv": _pack_dr(Wv_, SW),
            "bqk": np.ascontiguousarray(bqk_h, dtype=np.float32),
            "bv": np.clip(bv_ * 2048.0, -448.0, 448.0).reshape(1, DQ).astype(E4),
            "alib": np.ascontiguousarray(al_sb, dtype=np.float32),
            "Wd": np.ascontiguousarray(
                Wd[DQ * g:DQ * g + DQ, :].reshape(HPC, P, H).transpose(1, 0, 2)
            ).astype(BF),
            "masks": mask_host,
        })

    res = run_bass_kernel_spmd(nc, in_maps, list(range(NCORES)))

    outp = np.zeros((B, S, H), np.float32)
    for core in range(NCORES):
        outp[core // HPC] += res.results[core]["out"].astype(np.float32)
    outp += bd[None, None, :] + residual
    return outp


# revision 15
# speedup vs baseline: 1.0124x; 1.0124x over previous
"""BLOOM-style attention block (QKV proj + ALiBi causal attention + dense + residual)
for Trainium2, SPMD over 8 NeuronCores.

Sharding: core c -> (b = c // 4, head group g = c % 4, heads [4g..4g+4)).

v2: fp8 (e4m3) DoubleRow matmuls for the QKV projections and probs@V;
bf16 for scores and dense (error budget: ~1.6e-2 vs 2e-2 gate).

Per core (b, g):
  - hT packed as K=256 DoubleRow pairs [128, 8, 2, S] fp8 (x32 scale)
  - q/k proj DR (Wqk fp8 x1024): 8 DR matmuls per [128,512] tile; ACT applies
    scale (alpha/32768 for q, 1/32768 for k) + bias -> bf16 tiles
  - v proj DR -> +bias via ones(16)xfp8(bv*2048) matmul -> ACT x(1/1024) -> fp8 (32*v)
    stored as k-tile pairs v2 [128, 2, DQ]
  - scores bf16 transposed per (head, q-block): s_T [k128, q512]
  - additive -30000 mask (DVE) on partial tiles; exp via ACT with per-partition
    alibi+ln2 bias -> fp8 probs (2*p) written into pair tiles pt2 [128, 2, 512]
  - ctx DR (v2 x pt2) accumulated in PSUM; denominator via ones-DR matmul into
    a [1,512] PSUM accumulator (no DVE/gpsimd reduction)
  - rec = 1/d (DVE), broadcast via gpsimd, ctx normalized by DVE mul -> bf16
  - dense bf16 per q-block interleaved after each attn block; ACT copy x(1/32)
Host: out[b] = sum_g partial[b,g] + bd + residual[b].
"""

import numpy as np
import ml_dtypes

import concourse.bacc as bacc
import concourse.mybir as mybir
from concourse import tile
from concourse.bass_utils import run_bass_kernel_spmd

B, S, H, NH, HD = 2, 2048, 2048, 16, 128
ALPHA = 1.0 / float(np.sqrt(HD))
P = 128
NCORES = 8
HPC = 4            # heads per core
DQ = HPC * HD      # 512 = per-core q/k/v width
NJ = S // 512      # 4 q blocks of 512
NKT = S // P       # 16 k tiles of 128
NKH2 = H // 256    # 8 DoubleRow contraction pairs for projections
NCOL = 2 * DQ // P  # 8 qk col tiles (c<4: q head c, c>=4: k head c-4)
F32 = mybir.dt.float32
BF16 = mybir.dt.bfloat16
F8 = mybir.dt.float8e4
BF = ml_dtypes.bfloat16
E4 = ml_dtypes.float8_e4m3fn

SH = 32.0        # hidden fp8 scale
SW = 1024.0      # weight fp8 scale
SV = 32.0        # v fp8 scale (= SH*SW / 1024)
SP = 1.0         # prob fp8 scale (folded into alibi bias; 1.0: max unnorm prob ~167 << 448)
LN_SP = float(np.log(SP))

_cache = {}


def _analyze_mask(mask_b):
    """mask_b: [S, S] bool, True == masked out. Per (i, J) k/q tile:
    'skip' | pattern-index | None. Patterns are additive [k128, q512] f32."""
    patterns, pat_index, tilemap = [], {}, {}
    for J in range(NJ):
        for i in range(NKT):
            sub = mask_b[512 * J:512 * J + 512, P * i:P * i + P]  # [q, k]
            if sub.all():
                tilemap[(i, J)] = 'skip'
            elif not sub.any():
                tilemap[(i, J)] = None
            else:
                pat = np.where(sub.T, np.float32(-30000.0), np.float32(0.0))
                key = pat.tobytes()
                if key not in pat_index:
                    pat_index[key] = len(patterns)
                    patterns.append(pat)
                tilemap[(i, J)] = pat_index[key]
    return tilemap, patterns


def _build_program(tilemap, npat):
    nc = bacc.Bacc(None, target_bir_lowering=False, debug=False)

    hT = nc.dram_tensor("hT", [P, NJ, NKH2, 2, 512], F8, kind="ExternalInput")
    Wqk = nc.dram_tensor("Wqk", [P, 2, NKH2, 2, DQ], F8, kind="ExternalInput")
    Wv = nc.dram_tensor("Wv", [P, NKH2, 2, DQ], F8, kind="ExternalInput")
    bqk = nc.dram_tensor("bqk", [P, NCOL], F32, kind="ExternalInput")
    bv = nc.dram_tensor("bv", [1, DQ], F8, kind="ExternalInput")
    alib = nc.dram_tensor("alib", [P, HPC * NKT], F32, kind="ExternalInput")
    Wd = nc.dram_tensor("Wd", [P, HPC, H], BF16, kind="ExternalInput")
    masks = nc.dram_tensor("masks", [P, max(npat, 1) * 512], F32, kind="ExternalInput")
    out = nc.dram_tensor("out", [S, H], BF16, kind="ExternalOutput")

    with tile.TileContext(nc) as tc:
        with (
            tc.tile_pool(name="wqk", bufs=1) as wqk_pool,
            tc.tile_pool(name="wv", bufs=1) as wv_pool,
            tc.tile_pool(name="wd", bufs=1) as wd_pool,
            tc.tile_pool(name="consts", bufs=1) as consts,
            tc.tile_pool(name="hts", bufs=NJ) as ht_pool,
            tc.tile_pool(name="qkt", bufs=NCOL * NJ + 1) as qkt_pool,
            tc.tile_pool(name="vsb", bufs=NKT // 2 + 1) as v_pool,
            tc.tile_pool(name="prob", bufs=5) as prob_pool,
            tc.tile_pool(name="dstat", bufs=7) as dstat_pool,
            tc.tile_pool(name="ctxt", bufs=HPC * NJ + 1) as ctxt_pool,
            tc.tile_pool(name="ostage", bufs=3) as ostage_pool,
            tc.tile_pool(name="pp_proj", bufs=2, space="PSUM") as pp_proj,
            tc.tile_pool(name="pp_score", bufs=2, space="PSUM") as pp_score,
            tc.tile_pool(name="pp_ctx", bufs=2, space="PSUM") as pp_ctx,
            tc.tile_pool(name="pp_d", bufs=2, space="PSUM") as pp_d,
        ):
            # ---- constants ----
            mask_sb = consts.tile([P, max(npat, 1) * 512], F32)
            nc.sync.dma_start(out=mask_sb[:], in_=masks[:])
            al_sb = consts.tile([P, HPC * NKT], F32)
            nc.sync.dma_start(out=al_sb[:], in_=alib[:])
            bqk_sb = consts.tile([P, NCOL], F32)
            nc.sync.dma_start(out=bqk_sb[:], in_=bqk[:])
            bv_sb = consts.tile([1, DQ], F8)
            nc.sync.dma_start(out=bv_sb[:], in_=bv[:])
            ones16 = consts.tile([1, P], F8)
            nc.any.memset(ones16[:], 16.0)
            ones8 = consts.tile([P, 2, 32], F8)
            nc.any.memset(ones8[:], 1.0)

            # ---- batched input DMAs (issue cost ~0.66us each on the Sync
            # queue dominates with many small transfers): one DMA per hT
            # block, two for Wqk (lo/hi cols), one each for Wv / Wd ----
            ht_sb = {}  # (j, kh2) -> [128, 2, 512] fp8 view

            def load_ht(j):
                t = ht_pool.tile([P, NKH2, 2, 512], F8, tag="ht", name=f"htb{j}")
                nc.sync.dma_start(out=t[:], in_=hT[:, j, :, :, :])
                for kh2 in range(NKH2):
                    ht_sb[(j, kh2)] = t[:, kh2, :, :]

            load_ht(0)
            wqk_big = wqk_pool.tile([P, 2, NKH2, 2, DQ], F8, tag="wqk")
            nc.sync.dma_start(out=wqk_big[:, 0, :, :, :], in_=Wqk[:, 0, :, :, :])
            nc.sync.dma_start(out=wqk_big[:, 1, :, :, :], in_=Wqk[:, 1, :, :, :])

            def wqk_view(kh2, c):
                # c<4: q head c in the lo half; c>=4: k head c-4 in the hi half
                h, cc = (0, c) if c < HPC else (1, c - HPC)
                return wqk_big[:, h, kh2, :, P * cc:P * cc + P]
            wv_big = wv_pool.tile([P, NKH2, 2, DQ], F8, tag="wv")
            nc.sync.dma_start(out=wv_big[:], in_=Wv[:])
            wv_sb = [wv_big[:, kh2, :, :] for kh2 in range(NKH2)]
            load_ht(1)
            load_ht(2)
            load_ht(3)
            wd_big = wd_pool.tile([P, HPC, H], BF16, tag="wd")
            nc.sync.dma_start(out=wd_big[:], in_=Wd[:])
            wd_sb = {}  # (kd, cb) -> [128, 512] bf16 view
            for kd in range(HPC):
                for cb in range(NJ):
                    wd_sb[(kd, cb)] = wd_big[:, kd, 512 * cb:512 * cb + 512]

            qkt_sb = {}   # (c, j) -> [128, 512] bf16; c<4: q head c (alpha-scaled), c>=4: k
            v_sb = {}     # pr -> [128, 2, DQ] fp8 (32*v), k-tiles (2pr, 2pr+1)
            ctxt_sb = {}  # (h, J) -> [128, 512] bf16 (32*ctx)

            def proj_sweeps(j):
                sweeps = []

                def qk_sweep(cg, j=j):
                    ps = [pp_proj.tile([P, 512], F32, tag="ps", name=f"ps{j}_{cg}_{_i}")
                          for _i in range(2)]
                    for kh2 in range(NKH2):
                        for cc in range(2):
                            c = 2 * cg + cc
                            nc.tensor.matmul(
                                ps[cc][:],
                                wqk_view(kh2, c),
                                ht_sb[(j, kh2)][:],
                                start=(kh2 == 0), stop=(kh2 == NKH2 - 1),
                                perf_mode=mybir.MatmulPerfMode.DoubleRow,
                            )
                    for cc in range(2):
                        c = 2 * cg + cc
                        sc = (ALPHA / (SH * SW)) if c < HPC else (1.0 / (SH * SW))
                        qt = qkt_pool.tile([P, 512], BF16, tag="qkt")
                        nc.vector.tensor_scalar(
                            qt[:], ps[cc][:], sc, bqk_sb[:, c:c + 1],
                            mybir.AluOpType.mult, mybir.AluOpType.add)
                        qkt_sb[(c, j)] = qt

                def v_sweep(vg, j=j):
                    pv = [pp_proj.tile([P, DQ], F32, tag="ps", name=f"pv{j}_{vg}_{_i}")
                          for _i in range(2)]
                    for kh2 in range(NKH2):
                        for mm in range(2):
                            m = 2 * vg + mm
                            nc.tensor.matmul(
                                pv[mm][:],
                                ht_sb[(j, kh2)][:, :, P * m:P * m + P],
                                wv_sb[kh2][:],
                                start=(kh2 == 0), stop=False,
                                perf_mode=mybir.MatmulPerfMode.DoubleRow,
                            )
                    for mm in range(2):
                        m = 4 * j + 2 * vg + mm
                        nc.tensor.matmul(
                            pv[mm][:], ones16[:], bv_sb[:], start=False, stop=True)
                        if m % 2 == 0:
                            vt = v_pool.tile([P, 2, DQ], F8, tag="v",
                                             name=f"v{m // 2}")
                            v_sb[m // 2] = vt
                        nc.scalar.activation(
                            v_sb[m // 2][:, m % 2, :], pv[mm][:],
                            mybir.ActivationFunctionType.Identity,
                            scale=1.0 / SW)

                for cg in range(4):
                    sweeps.append(lambda cg=cg: qk_sweep(cg))
                for vg in range(2):
                    sweeps.append(lambda vg=vg: v_sweep(vg))
                return sweeps

            def attn_head(h, J, pctx, pd):
                """Emit one pair-step generator for head h, block J."""
                needed = [i for i in range(NKT) if tilemap[(i, J)] != 'skip']
                npair = (len(needed) + 1) // 2
                for pr in range(npair):
                    pt2 = prob_pool.tile([P, 2, 512], F8, tag="pt")
                    halves = needed[2 * pr:2 * pr + 2]
                    for half, i in enumerate(halves):
                        pscore = pp_score.tile([P, 512], F32, tag="pscore")
                        nc.tensor.matmul(
                            pscore[:],
                            qkt_sb[(HPC + h, i // 4)][:, P * (i % 4):P * (i % 4) + P],
                            qkt_sb[(h, J)][:],
                            start=True, stop=True,
                        )
                        pat = tilemap[(i, J)]
                        if pat is not None:
                            nc.vector.tensor_add(
                                pscore[:], pscore[:],
                                mask_sb[:, 512 * pat:512 * pat + 512])
                        nc.scalar.activation(
                            pt2[:, half, :], pscore[:],
                            mybir.ActivationFunctionType.Exp,
                            bias=al_sb[:, h * NKT + i:h * NKT + i + 1],
                        )
                    if len(halves) == 1:
                        nc.any.memset(pt2[:, 1, :], 0.0)
                    yield
                    nc.tensor.matmul(
                        pctx[:],
                        v_sb[needed[2 * pr] // 2][:, :, P * h:P * h + P],
                        pt2[:],
                        start=(pr == 0), stop=(pr == npair - 1),
                        perf_mode=mybir.MatmulPerfMode.DoubleRow,
                    )
                    nc.tensor.matmul(
                        pd[:], ones8[:], pt2[:],
                        start=(pr == 0), stop=(pr == npair - 1),
                        perf_mode=mybir.MatmulPerfMode.DoubleRow,
                    )
                    yield

            def finish_head(h, J, pctx, pd):
                deps = dstat_pool.tile([1, 512], F32, tag="deps")
                nc.vector.tensor_scalar_add(deps[:], pd[0:1, :], 1e-12)
                rec = dstat_pool.tile([1, 512], F32, tag="rec")
                nc.vector.reciprocal_approx_fast(rec[:], deps[:])
                recb = dstat_pool.tile([P, 512], F32, tag="recb")
                nc.gpsimd.partition_broadcast(recb[:], rec[:], 128)
                ct = ctxt_pool.tile([P, 512], BF16, tag="ctxt")
                nc.vector.tensor_mul(ct[:], pctx[:], recb[:])
                ctxt_sb[(h, J)] = ct

            def attn_block(J):
                """Generator: yields after each interleaved 2-head round."""
                for hp in range(2):
                    h0, h1 = 2 * hp, 2 * hp + 1
                    pctx0 = pp_ctx.tile([P, 512], F32, tag="pctx", name=f"pc{J}_{h0}")
                    pctx1 = pp_ctx.tile([P, 512], F32, tag="pctx", name=f"pc{J}_{h1}")
                    pd0 = pp_d.tile([32, 512], F32, tag="pd", name=f"pd{J}_{h0}")
                    pd1 = pp_d.tile([32, 512], F32, tag="pd", name=f"pd{J}_{h1}")
                    gens = [(attn_head(h0, J, pctx0, pd0), h0, pctx0, pd0),
                            (attn_head(h1, J, pctx1, pd1), h1, pctx1, pd1)]
                    live = list(gens)
                    while live:
                        for item in list(live):
                            g, h, pc, pdd = item
                            try:
                                next(g)
                            except StopIteration:
                                finish_head(h, J, pc, pdd)
                                live.remove(item)
                        yield

            def dense_groups(J, pools=None):
                groups = []
                pools = pools or [pp_proj]
                obig = {}

                def grp(cb, mm, J=J):
                    pool = pools[(4 * mm + cb) % len(pools)]
                    tg = "ps" if pool is pp_proj else "pscore"
                    m = 4 * J + mm
                    pdn = pool.tile([P, 512], F32, tag=tg, name=f"dn{J}_{cb}_{mm}")
                    for kd in range(HPC):
                        nc.tensor.matmul(
                            pdn[:],
                            ctxt_sb[(kd, J)][:, P * mm:P * mm + P],
                            wd_sb[(kd, cb)][:],
                            start=(kd == 0), stop=(kd == HPC - 1),
                        )
                    if mm not in obig:
                        obig[mm] = ostage_pool.tile(
                            [P, H], BF16, tag="ot", name=f"ob{J}_{mm}")
                    nc.vector.tensor_scalar_mul(
                        obig[mm][:, 512 * cb:512 * cb + 512], pdn[:], 1.0 / SV)
                    if cb == NJ - 1:
                        nc.sync.dma_start(
                            out=out[P * m:P * m + P, :], in_=obig[mm][:])

                for mm in range(4):
                    for cb in range(NJ):
                        groups.append(lambda cb=cb, mm=mm: grp(cb, mm))
                return groups

            # ---- master schedule: proj(j) sweeps interleaved with
            # attn(j-1) rounds; dense groups pumped into attn(3) bubbles ----
            dense_q = []

            def pump(n):
                while n > 0 and dense_q:
                    dense_q.pop(0)()
                    n -= 1

            for sw in proj_sweeps(0):
                sw()
            for j in range(1, NJ):
                rounds = attn_block(j - 1)
                sweeps = proj_sweeps(j)
                npair_prev = 2 * (j - 1) + 2
                nround = 2 * (2 * npair_prev + 1)  # rounds incl. finish
                per = max(1, (nround + len(sweeps) - 1) // len(sweeps))
                exhausted = False
                for sw in sweeps:
                    sw()
                    for _ in range(per):
                        try:
                            next(rounds)
                        except StopIteration:
                            exhausted = True
                            break
                    pump(1)
                while not exhausted:
                    try:
                        next(rounds)
                        pump(1)
                    except StopIteration:
                        exhausted = True
                dense_q.extend(dense_groups(j - 1))
            for _ in attn_block(NJ - 1):
                pump(3)
            dense_q.extend(dense_groups(NJ - 1, pools=[pp_proj, pp_score]))
            pump(len(dense_q))

    nc.finalize()
    return nc


def _pack_dr(mat, scale):
    """[H, C] f32 -> [128, NKH2, 2, C] fp8 DoubleRow pairs, scaled."""
    h, c = mat.shape
    m = np.clip(mat * scale, -448.0, 448.0).astype(E4)
    return np.ascontiguousarray(m.reshape(NKH2, 2, P, c).transpose(2, 0, 1, 3))


def kernel(hidden_states, attention_mask, residual, alibi, Wqkv, bqkv, Wd, bd):
    hidden_states = np.asarray(hidden_states, np.float32)
    attention_mask = np.asarray(attention_mask).astype(bool)
    residual = np.asarray(residual, np.float32)
    alibi = np.asarray(alibi, np.float32)
    Wqkv = np.asarray(Wqkv, np.float32)
    bqkv = np.asarray(bqkv, np.float32)
    Wd = np.asarray(Wd, np.float32)
    bd = np.asarray(bd, np.float32)

    m0 = attention_mask[0, 0]
    for b in range(1, B):
        assert np.array_equal(attention_mask[b, 0], m0), "per-batch masks differ"
    tilemap, patterns = _analyze_mask(m0)
    npat = len(patterns)
    assert npat <= 8, f"too many mask patterns: {npat}"
    mask_host = np.ascontiguousarray(
        np.concatenate(patterns, axis=1) if npat else np.zeros((P, 512), np.float32))

    key = tuple(sorted((k, str(v)) for k, v in tilemap.items())) + (npat,)
    if key not in _cache:
        _cache[key] = _build_program(tilemap, npat)
    nc = _cache[key]

    Wq3 = Wqkv.reshape(H, NH, 3, HD)   # col = nh*384 + {0:k,1:q,2:v}*128 + d
    bq3 = bqkv.reshape(NH, 3, HD)

    hT_cores = {}
    for b in range(B):
        p = _pack_dr(np.ascontiguousarray(hidden_states[b].T), SH)  # [P,NKH2,2,S]
        hT_cores[b] = np.ascontiguousarray(
            p.reshape(P, NKH2, 2, NJ, 512).transpose(0, 3, 1, 2, 4))

    in_maps = []
    for core in range(NCORES):
        b, g = divmod(core, HPC)
        hs = [HPC * g + t for t in range(HPC)]
        Wq = np.concatenate([Wq3[:, h, 1, :] for h in hs], 1)
        Wk = np.concatenate([Wq3[:, h, 0, :] for h in hs], 1)
        Wv_ = np.concatenate([Wq3[:, h, 2, :] for h in hs], 1)
        bq = np.concatenate([bq3[h, 1, :] for h in hs]) * ALPHA
        bk = np.concatenate([bq3[h, 0, :] for h in hs])
        bv_ = np.concatenate([bq3[h, 2, :] for h in hs])
        bqk_h = np.concatenate([bq, bk]).reshape(NCOL, P).T
        al_h = np.stack([alibi[b * NH + h, 0] for h in hs], 0) + LN_SP  # [HPC, S]
        al_sb = al_h.reshape(HPC, NKT, P).transpose(2, 0, 1).reshape(P, HPC * NKT)
        in_maps.append({
            "hT": hT_cores[b],
            "Wqk": np.ascontiguousarray(np.stack(
                [_pack_dr(Wq, SW), _pack_dr(Wk, SW)], axis=1)),
            "Wv": _pack_dr(Wv_, SW),
            "bqk": np.ascontiguousarray(bqk_h, dtype=np.float32),
            "bv": np.clip(bv_ * 2048.0, -448.0, 448.0).reshape(1, DQ).astype(E4),
            "alib": np.ascontiguousarray(al_sb, dtype=np.float32),
            "Wd": np.ascontiguousarray(
                Wd[DQ * g:DQ * g + DQ, :].reshape(HPC, P, H).transpose(1, 0, 2)
            ).astype(BF),
            "masks": mask_host,
        })

    res = run_bass_kernel_spmd(nc, in_maps, list(range(NCORES)))

    outp = np.zeros((B, S, H), np.float32)
    for core in range(NCORES):
        outp[core // HPC] += res.results[core]["out"].astype(np.float32)
    outp += bd[None, None, :] + residual
    return outp


# revision 16
# speedup vs baseline: 1.2060x; 1.1912x over previous
"""BLOOM-style attention block (QKV proj + ALiBi causal attention + dense + residual)
for Trainium2, SPMD over 8 NeuronCores.

Sharding: core c -> (b = c // 4, head group g = c % 4, heads [4g..4g+4)).

v2: fp8 (e4m3) DoubleRow matmuls for the QKV projections and probs@V;
bf16 for scores and dense (error budget: ~1.6e-2 vs 2e-2 gate).

Per core (b, g):
  - hT packed as K=256 DoubleRow pairs [128, 8, 2, S] fp8 (x32 scale)
  - q/k proj DR (Wqk fp8 x1024): 8 DR matmuls per [128,512] tile; ACT applies
    scale (alpha/32768 for q, 1/32768 for k) + bias -> bf16 tiles
  - v proj DR -> +bias via ones(16)xfp8(bv*2048) matmul -> ACT x(1/1024) -> fp8 (32*v)
    stored as k-tile pairs v2 [128, 2, DQ]
  - scores bf16 transposed per (head, q-block): s_T [k128, q512]
  - additive -30000 mask (DVE) on partial tiles; exp via ACT with per-partition
    alibi+ln2 bias -> fp8 probs (2*p) written into pair tiles pt2 [128, 2, 512]
  - ctx DR (v2 x pt2) accumulated in PSUM; denominator via ones-DR matmul into
    a [1,512] PSUM accumulator (no DVE/gpsimd reduction)
  - rec = 1/d (DVE), broadcast via gpsimd, ctx normalized by DVE mul -> bf16
  - dense bf16 per q-block interleaved after each attn block; ACT copy x(1/32)
Host: out[b] = sum_g partial[b,g] + bd + residual[b].
"""

import numpy as np
import ml_dtypes

import concourse.bacc as bacc
import concourse.mybir as mybir
from concourse import tile
from concourse.bass_utils import run_bass_kernel_spmd

B, S, H, NH, HD = 2, 2048, 2048, 16, 128
ALPHA = 1.0 / float(np.sqrt(HD))
P = 128
NCORES = 8
HPC = 4            # heads per core
DQ = HPC * HD      # 512 = per-core q/k/v width
NJ = S // 512      # 4 q blocks of 512
NKT = S // P       # 16 k tiles of 128
NKH2 = H // 256    # 8 DoubleRow contraction pairs for projections
NCOL = 2 * DQ // P  # 8 qk col tiles (c<4: q head c, c>=4: k head c-4)
F32 = mybir.dt.float32
BF16 = mybir.dt.bfloat16
F8 = mybir.dt.float8e4
BF = ml_dtypes.bfloat16
E4 = ml_dtypes.float8_e4m3fn

SH = 32.0        # hidden fp8 scale
SW = 1024.0      # weight fp8 scale
SV = 32.0        # v fp8 scale (= SH*SW / 1024)
SP = 1.0         # prob fp8 scale (folded into alibi bias; 1.0: max unnorm prob ~167 << 448)
LN_SP = float(np.log(SP))

_cache = {}


def _analyze_mask(mask_b):
    """mask_b: [S, S] bool, True == masked out. Per (i, J) k/q tile:
    'skip' | pattern-index | None. Patterns are additive [k128, q512] f32."""
    patterns, pat_index, tilemap = [], {}, {}
    for J in range(NJ):
        for i in range(NKT):
            sub = mask_b[512 * J:512 * J + 512, P * i:P * i + P]  # [q, k]
            if sub.all():
                tilemap[(i, J)] = 'skip'
            elif not sub.any():
                tilemap[(i, J)] = None
            else:
                pat = np.where(sub.T, np.float32(-30000.0), np.float32(0.0))
                key = pat.tobytes()
                if key not in pat_index:
                    pat_index[key] = len(patterns)
                    patterns.append(pat)
                tilemap[(i, J)] = pat_index[key]
    return tilemap, patterns


def _build_program(tilemap, npat, wins):
    # wins: per head-slot, tuple of kept k-tile indices (alibi window)
    nc = bacc.Bacc(None, target_bir_lowering=False, debug=False)

    hT = nc.dram_tensor("hT", [P, NJ, NKH2, 2, 512], F8, kind="ExternalInput")
    Wqk = nc.dram_tensor("Wqk", [P, 2, NKH2, 2, DQ], F8, kind="ExternalInput")
    Wv = nc.dram_tensor("Wv", [P, NKH2, 2, DQ], F8, kind="ExternalInput")
    bqk = nc.dram_tensor("bqk", [P, NCOL], F32, kind="ExternalInput")
    bv = nc.dram_tensor("bv", [1, DQ], F8, kind="ExternalInput")
    alib = nc.dram_tensor("alib", [P, HPC * NKT], F32, kind="ExternalInput")
    Wd = nc.dram_tensor("Wd", [P, HPC, H], BF16, kind="ExternalInput")
    masks = nc.dram_tensor("masks", [P, max(npat, 1) * 512], F32, kind="ExternalInput")
    out = nc.dram_tensor("out", [S, H], BF16, kind="ExternalOutput")

    with tile.TileContext(nc) as tc:
        with (
            tc.tile_pool(name="wqk", bufs=1) as wqk_pool,
            tc.tile_pool(name="wv", bufs=1) as wv_pool,
            tc.tile_pool(name="wd", bufs=1) as wd_pool,
            tc.tile_pool(name="consts", bufs=1) as consts,
            tc.tile_pool(name="hts", bufs=NJ) as ht_pool,
            tc.tile_pool(name="qkt", bufs=NCOL * NJ + 1) as qkt_pool,
            tc.tile_pool(name="vsb", bufs=NKT // 2 + 1) as v_pool,
            tc.tile_pool(name="prob", bufs=5) as prob_pool,
            tc.tile_pool(name="dstat", bufs=7) as dstat_pool,
            tc.tile_pool(name="ctxt", bufs=HPC * NJ + 1) as ctxt_pool,
            tc.tile_pool(name="ostage", bufs=3) as ostage_pool,
            tc.tile_pool(name="pp_proj", bufs=2, space="PSUM") as pp_proj,
            tc.tile_pool(name="pp_score", bufs=2, space="PSUM") as pp_score,
            tc.tile_pool(name="pp_ctx", bufs=2, space="PSUM") as pp_ctx,
            tc.tile_pool(name="pp_d", bufs=2, space="PSUM") as pp_d,
        ):
            # ---- constants ----
            mask_sb = consts.tile([P, max(npat, 1) * 512], F32)
            nc.sync.dma_start(out=mask_sb[:], in_=masks[:])
            al_sb = consts.tile([P, HPC * NKT], F32)
            nc.sync.dma_start(out=al_sb[:], in_=alib[:])
            bqk_sb = consts.tile([P, NCOL], F32)
            nc.sync.dma_start(out=bqk_sb[:], in_=bqk[:])
            bv_sb = consts.tile([1, DQ], F8)
            nc.sync.dma_start(out=bv_sb[:], in_=bv[:])
            ones16 = consts.tile([1, P], F8)
            nc.any.memset(ones16[:], 16.0)
            ones8 = consts.tile([P, 2, 32], F8)
            nc.any.memset(ones8[:], 1.0)

            # ---- batched input DMAs (issue cost ~0.66us each on the Sync
            # queue dominates with many small transfers): one DMA per hT
            # block, two for Wqk (lo/hi cols), one each for Wv / Wd ----
            ht_sb = {}  # (j, kh2) -> [128, 2, 512] fp8 view

            def load_ht(j):
                t = ht_pool.tile([P, NKH2, 2, 512], F8, tag="ht", name=f"htb{j}")
                nc.sync.dma_start(out=t[:], in_=hT[:, j, :, :, :])
                for kh2 in range(NKH2):
                    ht_sb[(j, kh2)] = t[:, kh2, :, :]

            load_ht(0)
            wqk_big = wqk_pool.tile([P, 2, NKH2, 2, DQ], F8, tag="wqk")
            nc.sync.dma_start(out=wqk_big[:, 0, :, :, :], in_=Wqk[:, 0, :, :, :])
            nc.sync.dma_start(out=wqk_big[:, 1, :, :, :], in_=Wqk[:, 1, :, :, :])

            def wqk_view(kh2, c):
                # c<4: q head c in the lo half; c>=4: k head c-4 in the hi half
                h, cc = (0, c) if c < HPC else (1, c - HPC)
                return wqk_big[:, h, kh2, :, P * cc:P * cc + P]
            wv_big = wv_pool.tile([P, NKH2, 2, DQ], F8, tag="wv")
            nc.sync.dma_start(out=wv_big[:], in_=Wv[:])
            wv_sb = [wv_big[:, kh2, :, :] for kh2 in range(NKH2)]
            load_ht(1)
            load_ht(2)
            load_ht(3)
            wd_big = wd_pool.tile([P, HPC, H], BF16, tag="wd")
            nc.sync.dma_start(out=wd_big[:], in_=Wd[:])
            wd_sb = {}  # (kd, cb) -> [128, 512] bf16 view
            for kd in range(HPC):
                for cb in range(NJ):
                    wd_sb[(kd, cb)] = wd_big[:, kd, 512 * cb:512 * cb + 512]

            qkt_sb = {}   # (c, j) -> [128, 512] bf16; c<4: q head c (alpha-scaled), c>=4: k
            v_sb = {}     # pr -> [128, 2, DQ] fp8 (32*v), k-tiles (2pr, 2pr+1)
            ctxt_sb = {}  # (h, J) -> [128, 512] bf16 (32*ctx)

            def proj_sweeps(j):
                sweeps = []

                def qk_sweep(cg, j=j):
                    ps = [pp_proj.tile([P, 512], F32, tag="ps", name=f"ps{j}_{cg}_{_i}")
                          for _i in range(2)]
                    for kh2 in range(NKH2):
                        for cc in range(2):
                            c = 2 * cg + cc
                            nc.tensor.matmul(
                                ps[cc][:],
                                wqk_view(kh2, c),
                                ht_sb[(j, kh2)][:],
                                start=(kh2 == 0), stop=(kh2 == NKH2 - 1),
                                perf_mode=mybir.MatmulPerfMode.DoubleRow,
                            )
                    for cc in range(2):
                        c = 2 * cg + cc
                        sc = (ALPHA / (SH * SW)) if c < HPC else (1.0 / (SH * SW))
                        qt = qkt_pool.tile([P, 512], BF16, tag="qkt")
                        nc.vector.tensor_scalar(
                            qt[:], ps[cc][:], sc, bqk_sb[:, c:c + 1],
                            mybir.AluOpType.mult, mybir.AluOpType.add)
                        qkt_sb[(c, j)] = qt

                def v_sweep(vg, j=j):
                    pv = [pp_proj.tile([P, DQ], F32, tag="ps", name=f"pv{j}_{vg}_{_i}")
                          for _i in range(2)]
                    for kh2 in range(NKH2):
                        for mm in range(2):
                            m = 2 * vg + mm
                            nc.tensor.matmul(
                                pv[mm][:],
                                ht_sb[(j, kh2)][:, :, P * m:P * m + P],
                                wv_sb[kh2][:],
                                start=(kh2 == 0), stop=False,
                                perf_mode=mybir.MatmulPerfMode.DoubleRow,
                            )
                    for mm in range(2):
                        m = 4 * j + 2 * vg + mm
                        nc.tensor.matmul(
                            pv[mm][:], ones16[:], bv_sb[:], start=False, stop=True)
                        if m % 2 == 0:
                            vt = v_pool.tile([P, 2, DQ], F8, tag="v",
                                             name=f"v{m // 2}")
                            v_sb[m // 2] = vt
                        nc.scalar.activation(
                            v_sb[m // 2][:, m % 2, :], pv[mm][:],
                            mybir.ActivationFunctionType.Identity,
                            scale=1.0 / SW)

                for cg in range(4):
                    sweeps.append(lambda cg=cg: qk_sweep(cg))
                for vg in range(2):
                    sweeps.append(lambda vg=vg: v_sweep(vg))
                return sweeps

            def needed_tiles(h, J):
                w = set(wins[h])
                nd = [i for i in range(NKT) if tilemap[(i, J)] != 'skip' and i in w]
                assert nd == list(range(len(nd))), "window must be a prefix"
                return nd

            def attn_head(h, J, pctx, pd):
                """Emit one pair-step generator for head h, block J."""
                needed = needed_tiles(h, J)
                npair = (len(needed) + 1) // 2
                for pr in range(npair):
                    pt2 = prob_pool.tile([P, 2, 512], F8, tag="pt")
                    halves = needed[2 * pr:2 * pr + 2]
                    for half, i in enumerate(halves):
                        pscore = pp_score.tile([P, 512], F32, tag="pscore")
                        nc.tensor.matmul(
                            pscore[:],
                            qkt_sb[(HPC + h, i // 4)][:, P * (i % 4):P * (i % 4) + P],
                            qkt_sb[(h, J)][:],
                            start=True, stop=True,
                        )
                        pat = tilemap[(i, J)]
                        if pat is not None:
                            nc.vector.tensor_add(
                                pscore[:], pscore[:],
                                mask_sb[:, 512 * pat:512 * pat + 512])
                        nc.scalar.activation(
                            pt2[:, half, :], pscore[:],
                            mybir.ActivationFunctionType.Exp,
                            bias=al_sb[:, h * NKT + i:h * NKT + i + 1],
                        )
                    if len(halves) == 1:
                        nc.any.memset(pt2[:, 1, :], 0.0)
                    yield
                    nc.tensor.matmul(
                        pctx[:],
                        v_sb[needed[2 * pr] // 2][:, :, P * h:P * h + P],
                        pt2[:],
                        start=(pr == 0), stop=(pr == npair - 1),
                        perf_mode=mybir.MatmulPerfMode.DoubleRow,
                    )
                    nc.tensor.matmul(
                        pd[:], ones8[:], pt2[:],
                        start=(pr == 0), stop=(pr == npair - 1),
                        perf_mode=mybir.MatmulPerfMode.DoubleRow,
                    )
                    yield

            def finish_head(h, J, pctx, pd):
                deps = dstat_pool.tile([1, 512], F32, tag="deps")
                nc.vector.tensor_scalar_add(deps[:], pd[0:1, :], 1e-12)
                rec = dstat_pool.tile([1, 512], F32, tag="rec")
                nc.vector.reciprocal_approx_fast(rec[:], deps[:])
                recb = dstat_pool.tile([P, 512], F32, tag="recb")
                nc.gpsimd.partition_broadcast(recb[:], rec[:], 128)
                ct = ctxt_pool.tile([P, 512], BF16, tag="ctxt")
                nc.vector.tensor_mul(ct[:], pctx[:], recb[:])
                ctxt_sb[(h, J)] = ct

            def attn_block(J):
                """Generator: yields after each interleaved 2-head round."""
                for hp in range(2):
                    h0, h1 = 2 * hp, 2 * hp + 1
                    pctx0 = pp_ctx.tile([P, 512], F32, tag="pctx", name=f"pc{J}_{h0}")
                    pctx1 = pp_ctx.tile([P, 512], F32, tag="pctx", name=f"pc{J}_{h1}")
                    pd0 = pp_d.tile([32, 512], F32, tag="pd", name=f"pd{J}_{h0}")
                    pd1 = pp_d.tile([32, 512], F32, tag="pd", name=f"pd{J}_{h1}")
                    gens = [(attn_head(h0, J, pctx0, pd0), h0, pctx0, pd0),
                            (attn_head(h1, J, pctx1, pd1), h1, pctx1, pd1)]
                    live = list(gens)
                    while live:
                        for item in list(live):
                            g, h, pc, pdd = item
                            try:
                                next(g)
                            except StopIteration:
                                finish_head(h, J, pc, pdd)
                                live.remove(item)
                        yield

            def dense_groups(J, pools=None):
                groups = []
                pools = pools or [pp_proj]
                obig = {}

                def grp(cb, mm, J=J):
                    pool = pools[(4 * mm + cb) % len(pools)]
                    tg = "ps" if pool is pp_proj else "pscore"
                    m = 4 * J + mm
                    pdn = pool.tile([P, 512], F32, tag=tg, name=f"dn{J}_{cb}_{mm}")
                    for kd in range(HPC):
                        nc.tensor.matmul(
                            pdn[:],
                            ctxt_sb[(kd, J)][:, P * mm:P * mm + P],
                            wd_sb[(kd, cb)][:],
                            start=(kd == 0), stop=(kd == HPC - 1),
                        )
                    if mm not in obig:
                        obig[mm] = ostage_pool.tile(
                            [P, H], BF16, tag="ot", name=f"ob{J}_{mm}")
                    nc.vector.tensor_scalar_mul(
                        obig[mm][:, 512 * cb:512 * cb + 512], pdn[:], 1.0 / SV)
                    if cb == NJ - 1:
                        nc.sync.dma_start(
                            out=out[P * m:P * m + P, :], in_=obig[mm][:])

                for mm in range(4):
                    for cb in range(NJ):
                        groups.append(lambda cb=cb, mm=mm: grp(cb, mm))
                return groups

            # ---- master schedule: proj(j) sweeps interleaved with
            # attn(j-1) rounds; dense groups pumped into attn(3) bubbles ----
            dense_q = []

            def pump(n):
                while n > 0 and dense_q:
                    dense_q.pop(0)()
                    n -= 1

            for sw in proj_sweeps(0):
                sw()
            for j in range(1, NJ):
                rounds = attn_block(j - 1)
                sweeps = proj_sweeps(j)
                nround = 0
                for ha, hb in ((0, 1), (2, 3)):
                    npa = (len(needed_tiles(ha, j - 1)) + 1) // 2
                    npb = (len(needed_tiles(hb, j - 1)) + 1) // 2
                    nround += 2 * max(npa, npb) + 2
                per = max(1, (nround + len(sweeps) - 1) // len(sweeps))
                exhausted = False
                for sw in sweeps:
                    sw()
                    for _ in range(per):
                        try:
                            next(rounds)
                        except StopIteration:
                            exhausted = True
                            break
                    pump(1)
                while not exhausted:
                    try:
                        next(rounds)
                        pump(1)
                    except StopIteration:
                        exhausted = True
                dense_q.extend(dense_groups(j - 1))
            for _ in attn_block(NJ - 1):
                pump(3)
            dense_q.extend(dense_groups(NJ - 1, pools=[pp_proj, pp_score]))
            pump(len(dense_q))

    nc.finalize()
    return nc


def _pack_dr(mat, scale):
    """[H, C] f32 -> [128, NKH2, 2, C] fp8 DoubleRow pairs, scaled."""
    h, c = mat.shape
    m = np.clip(mat * scale, -448.0, 448.0).astype(E4)
    return np.ascontiguousarray(m.reshape(NKH2, 2, P, c).transpose(2, 0, 1, 3))


def kernel(hidden_states, attention_mask, residual, alibi, Wqkv, bqkv, Wd, bd):
    hidden_states = np.asarray(hidden_states, np.float32)
    attention_mask = np.asarray(attention_mask).astype(bool)
    residual = np.asarray(residual, np.float32)
    alibi = np.asarray(alibi, np.float32)
    Wqkv = np.asarray(Wqkv, np.float32)
    bqkv = np.asarray(bqkv, np.float32)
    Wd = np.asarray(Wd, np.float32)
    bd = np.asarray(bd, np.float32)

    m0 = attention_mask[0, 0]
    for b in range(1, B):
        assert np.array_equal(attention_mask[b, 0], m0), "per-batch masks differ"
    tilemap, patterns = _analyze_mask(m0)
    npat = len(patterns)
    assert npat <= 8, f"too many mask patterns: {npat}"
    mask_host = np.ascontiguousarray(
        np.concatenate(patterns, axis=1) if npat else np.zeros((P, 512), np.float32))

    # per-head alibi windows: keep k-tile i iff max alibi in tile >= -THR.
    # Skipped tiles have unnormalized probs <= e^(s_max - THR) ~ e^-22 -> 0.
    THR = 28.0
    keep = []
    for h in range(NH):
        km = tuple(
            bool(max(np.max(alibi[b * NH + h, 0, P * i:P * i + P]) for b in range(B))
                 >= -THR)
            for i in range(NKT))
        keep.append(km)
    order = sorted(range(NH), key=lambda h: sum(keep[h]))
    slots = [order[HPC * t:HPC * t + HPC] for t in range(HPC)]
    wins = tuple(
        tuple(i for i in range(NKT) if any(keep[h][i] for h in sl))
        for sl in slots)

    key = tuple(sorted((k, str(v)) for k, v in tilemap.items())) + (npat, wins)
    if key not in _cache:
        _cache[key] = _build_program(tilemap, npat, wins)
    nc = _cache[key]

    Wq3 = Wqkv.reshape(H, NH, 3, HD)   # col = nh*384 + {0:k,1:q,2:v}*128 + d
    bq3 = bqkv.reshape(NH, 3, HD)

    hT_cores = {}
    for b in range(B):
        p = _pack_dr(np.ascontiguousarray(hidden_states[b].T), SH)  # [P,NKH2,2,S]
        hT_cores[b] = np.ascontiguousarray(
            p.reshape(P, NKH2, 2, NJ, 512).transpose(0, 3, 1, 2, 4))

    in_maps = []
    for core in range(NCORES):
        b, g = divmod(core, HPC)
        hs = [slots[t][g] for t in range(HPC)]
        Wq = np.concatenate([Wq3[:, h, 1, :] for h in hs], 1)
        Wk = np.concatenate([Wq3[:, h, 0, :] for h in hs], 1)
        Wv_ = np.concatenate([Wq3[:, h, 2, :] for h in hs], 1)
        bq = np.concatenate([bq3[h, 1, :] for h in hs]) * ALPHA
        bk = np.concatenate([bq3[h, 0, :] for h in hs])
        bv_ = np.concatenate([bq3[h, 2, :] for h in hs])
        bqk_h = np.concatenate([bq, bk]).reshape(NCOL, P).T
        al_h = np.stack([alibi[b * NH + h, 0] for h in hs], 0) + LN_SP  # [HPC, S]
        al_sb = al_h.reshape(HPC, NKT, P).transpose(2, 0, 1).reshape(P, HPC * NKT)
        in_maps.append({
            "hT": hT_cores[b],
            "Wqk": np.ascontiguousarray(np.stack(
                [_pack_dr(Wq, SW), _pack_dr(Wk, SW)], axis=1)),
            "Wv": _pack_dr(Wv_, SW),
            "bqk": np.ascontiguousarray(bqk_h, dtype=np.float32),
            "bv": np.clip(bv_ * 2048.0, -448.0, 448.0).reshape(1, DQ).astype(E4),
            "alib": np.ascontiguousarray(al_sb, dtype=np.float32),
            "Wd": np.ascontiguousarray(
                np.stack([Wd[h * HD:(h + 1) * HD, :] for h in hs], 0)
                .transpose(1, 0, 2)).astype(BF),
            "masks": mask_host,
        })

    res = run_bass_kernel_spmd(nc, in_maps, list(range(NCORES)))

    outp = np.zeros((B, S, H), np.float32)
    for core in range(NCORES):
        outp[core // HPC] += res.results[core]["out"].astype(np.float32)
    outp += bd[None, None, :] + residual
    return outp


# revision 17
# speedup vs baseline: 1.3299x; 1.1027x over previous
"""BLOOM-style attention block (QKV proj + ALiBi causal attention + dense + residual)
for Trainium2, SPMD over 8 NeuronCores.

Sharding: core c -> (b = c // 4, head group g = c % 4, heads [4g..4g+4)).

v2: fp8 (e4m3) DoubleRow matmuls for the QKV projections and probs@V;
bf16 for scores and dense (error budget: ~1.6e-2 vs 2e-2 gate).

Per core (b, g):
  - hT packed as K=256 DoubleRow pairs [128, 8, 2, S] fp8 (x32 scale)
  - q/k proj DR (Wqk fp8 x1024): 8 DR matmuls per [128,512] tile; ACT applies
    scale (alpha/32768 for q, 1/32768 for k) + bias -> bf16 tiles
  - v proj DR -> +bias via ones(16)xfp8(bv*2048) matmul -> ACT x(1/1024) -> fp8 (32*v)
    stored as k-tile pairs v2 [128, 2, DQ]
  - scores bf16 transposed per (head, q-block): s_T [k128, q512]
  - additive -30000 mask (DVE) on partial tiles; exp via ACT with per-partition
    alibi+ln2 bias -> fp8 probs (2*p) written into pair tiles pt2 [128, 2, 512]
  - ctx DR (v2 x pt2) accumulated in PSUM; denominator via ones-DR matmul into
    a [1,512] PSUM accumulator (no DVE/gpsimd reduction)
  - rec = 1/d (DVE), broadcast via gpsimd, ctx normalized by DVE mul -> bf16
  - dense bf16 per q-block interleaved after each attn block; ACT copy x(1/32)
Host: out[b] = sum_g partial[b,g] + bd + residual[b].
"""

import numpy as np
import ml_dtypes

import concourse.bacc as bacc
import concourse.mybir as mybir
from concourse import tile
from concourse.bass_utils import run_bass_kernel_spmd

B, S, H, NH, HD = 2, 2048, 2048, 16, 128
ALPHA = 1.0 / float(np.sqrt(HD))
P = 128
NCORES = 8
HPC = 4            # heads per core
DQ = HPC * HD      # 512 = per-core q/k/v width
NJ = S // 512      # 4 q blocks of 512
NKT = S // P       # 16 k tiles of 128
NKH2 = H // 256    # 8 DoubleRow contraction pairs for projections
NCOL = 2 * DQ // P  # 8 qk col tiles (c<4: q head c, c>=4: k head c-4)
F32 = mybir.dt.float32
BF16 = mybir.dt.bfloat16
F8 = mybir.dt.float8e4
BF = ml_dtypes.bfloat16
E4 = ml_dtypes.float8_e4m3fn

SH = 32.0        # hidden fp8 scale
SW = 1024.0      # weight fp8 scale
SV = 32.0        # v fp8 scale (= SH*SW / 1024)
SP = 1.0         # prob fp8 scale (folded into alibi bias; 1.0: max unnorm prob ~167 << 448)
LN_SP = float(np.log(SP))

_cache = {}


def _analyze_mask(mask_b):
    """mask_b: [S, S] bool, True == masked out. Per (i, J) k/q tile:
    'skip' | pattern-index | None. Patterns are additive [k128, q512] f32."""
    patterns, pat_index, tilemap = [], {}, {}
    for J in range(NJ):
        for i in range(NKT):
            sub = mask_b[512 * J:512 * J + 512, P * i:P * i + P]  # [q, k]
            if sub.all():
                tilemap[(i, J)] = 'skip'
            elif not sub.any():
                tilemap[(i, J)] = None
            else:
                pat = np.where(sub.T, np.float32(-30000.0), np.float32(0.0))
                key = pat.tobytes()
                if key not in pat_index:
                    pat_index[key] = len(patterns)
                    patterns.append(pat)
                tilemap[(i, J)] = pat_index[key]
    return tilemap, patterns


def _build_program(tilemap, npat, wins):
    # wins: per head-slot, tuple of kept k-tile indices (alibi window)
    nc = bacc.Bacc(None, target_bir_lowering=False, debug=False)

    hT = nc.dram_tensor("hT", [P, NJ, NKH2, 2, 512], F8, kind="ExternalInput")
    Wqk = nc.dram_tensor("Wqk", [P, 2, NKH2, 2, DQ], F8, kind="ExternalInput")
    Wv = nc.dram_tensor("Wv", [P, NKH2, 2, DQ], F8, kind="ExternalInput")
    bqk = nc.dram_tensor("bqk", [P, NCOL], F32, kind="ExternalInput")
    bv = nc.dram_tensor("bv", [1, DQ], F8, kind="ExternalInput")
    alib = nc.dram_tensor("alib", [P, HPC * NKT], F32, kind="ExternalInput")
    Wd = nc.dram_tensor("Wd", [P, HPC, H], BF16, kind="ExternalInput")
    masks = nc.dram_tensor("masks", [P, max(npat, 1) * 512], F32, kind="ExternalInput")
    out = nc.dram_tensor("out", [S, H], BF16, kind="ExternalOutput")

    with tile.TileContext(nc) as tc:
        with (
            tc.tile_pool(name="wqk", bufs=1) as wqk_pool,
            tc.tile_pool(name="wv", bufs=1) as wv_pool,
            tc.tile_pool(name="wd", bufs=1) as wd_pool,
            tc.tile_pool(name="consts", bufs=1) as consts,
            tc.tile_pool(name="hts", bufs=NJ) as ht_pool,
            tc.tile_pool(name="qkt", bufs=NCOL * NJ + 1) as qkt_pool,
            tc.tile_pool(name="vsb", bufs=NKT // 2 + 1) as v_pool,
            tc.tile_pool(name="prob", bufs=5) as prob_pool,
            tc.tile_pool(name="dstat", bufs=7) as dstat_pool,
            tc.tile_pool(name="ctxt", bufs=HPC * NJ + 1) as ctxt_pool,
            tc.tile_pool(name="ostage", bufs=3) as ostage_pool,
            tc.tile_pool(name="pp_proj", bufs=2, space="PSUM") as pp_proj,
            tc.tile_pool(name="pp_score", bufs=2, space="PSUM") as pp_score,
            tc.tile_pool(name="pp_ctx", bufs=2, space="PSUM") as pp_ctx,
            tc.tile_pool(name="pp_d", bufs=2, space="PSUM") as pp_d,
        ):
            # ---- constants ----
            mask_sb = consts.tile([P, max(npat, 1) * 512], F32)
            nc.sync.dma_start(out=mask_sb[:], in_=masks[:])
            al_sb = consts.tile([P, HPC * NKT], F32)
            nc.sync.dma_start(out=al_sb[:], in_=alib[:])
            bqk_sb = consts.tile([P, NCOL], F32)
            nc.sync.dma_start(out=bqk_sb[:], in_=bqk[:])
            bv_sb = consts.tile([1, DQ], F8)
            nc.sync.dma_start(out=bv_sb[:], in_=bv[:])
            ones16 = consts.tile([1, P], F8)
            nc.any.memset(ones16[:], 16.0)
            ones8 = consts.tile([P, 2, 32], F8)
            nc.any.memset(ones8[:], 1.0)

            # ---- batched input DMAs (issue cost ~0.66us each on the Sync
            # queue dominates with many small transfers): one DMA per hT
            # block, two for Wqk (lo/hi cols), one each for Wv / Wd ----
            ht_sb = {}  # (j, kh2) -> [128, 2, 512] fp8 view

            def load_ht(j):
                t = ht_pool.tile([P, NKH2, 2, 512], F8, tag="ht", name=f"htb{j}")
                nc.sync.dma_start(out=t[:], in_=hT[:, j, :, :, :])
                for kh2 in range(NKH2):
                    ht_sb[(j, kh2)] = t[:, kh2, :, :]

            t0 = ht_pool.tile([P, NKH2, 2, 512], F8, tag="ht", name="htb0")
            wqk_big = wqk_pool.tile([P, 2, NKH2, 2, DQ], F8, tag="wqk")
            hh = NKH2 // 2
            nc.sync.dma_start(out=t0[:, 0:hh, :, :], in_=hT[:, 0, 0:hh, :, :])
            nc.sync.dma_start(out=wqk_big[:, 0, 0:hh, :, :], in_=Wqk[:, 0, 0:hh, :, :])
            nc.sync.dma_start(out=t0[:, hh:NKH2, :, :], in_=hT[:, 0, hh:NKH2, :, :])
            nc.sync.dma_start(
                out=wqk_big[:, 0, hh:NKH2, :, :], in_=Wqk[:, 0, hh:NKH2, :, :])
            for kh2 in range(NKH2):
                ht_sb[(0, kh2)] = t0[:, kh2, :, :]
            nc.sync.dma_start(out=wqk_big[:, 1, :, :, :], in_=Wqk[:, 1, :, :, :])

            def wqk_view(kh2, c):
                # c<4: q head c in the lo half; c>=4: k head c-4 in the hi half
                h, cc = (0, c) if c < HPC else (1, c - HPC)
                return wqk_big[:, h, kh2, :, P * cc:P * cc + P]
            wv_big = wv_pool.tile([P, NKH2, 2, DQ], F8, tag="wv")
            nc.sync.dma_start(out=wv_big[:], in_=Wv[:])
            wv_sb = [wv_big[:, kh2, :, :] for kh2 in range(NKH2)]
            load_ht(1)
            load_ht(2)
            load_ht(3)
            wd_big = wd_pool.tile([P, HPC, H], BF16, tag="wd")
            nc.sync.dma_start(out=wd_big[:], in_=Wd[:])
            wd_sb = {}  # (kd, cb) -> [128, 512] bf16 view
            for kd in range(HPC):
                for cb in range(NJ):
                    wd_sb[(kd, cb)] = wd_big[:, kd, 512 * cb:512 * cb + 512]

            qkt_sb = {}   # (c, j) -> [128, 512] bf16; c<4: q head c (alpha-scaled), c>=4: k
            v_sb = {}     # pr -> [128, 2, DQ] fp8 (32*v), k-tiles (2pr, 2pr+1)
            ctxt_sb = {}  # (h, J) -> [128, 512] bf16 (32*ctx)

            def kslots(j):
                # head-slots whose alibi window reaches key block j
                return [t for t in range(HPC) if wins[t] and wins[t][-1] >= 4 * j]

            def proj_sweeps(j):
                sweeps = []
                cols = list(range(HPC)) + [HPC + t for t in kslots(j)]
                vlo = min(kslots(j)) * P
                w = DQ - vlo

                def qk_sweep(cpair, j=j):
                    ps = [pp_proj.tile([P, 512], F32, tag="ps",
                                       name=f"ps{j}_{c}") for c in cpair]
                    for kh2 in range(NKH2):
                        for cc, c in enumerate(cpair):
                            nc.tensor.matmul(
                                ps[cc][:],
                                wqk_view(kh2, c),
                                ht_sb[(j, kh2)][:],
                                start=(kh2 == 0), stop=(kh2 == NKH2 - 1),
                                perf_mode=mybir.MatmulPerfMode.DoubleRow,
                            )
                    for cc, c in enumerate(cpair):
                        sc = (ALPHA / (SH * SW)) if c < HPC else (1.0 / (SH * SW))
                        qt = qkt_pool.tile([P, 512], BF16, tag="qkt")
                        nc.vector.tensor_scalar(
                            qt[:], ps[cc][:], sc, bqk_sb[:, c:c + 1],
                            mybir.AluOpType.mult, mybir.AluOpType.add)
                        qkt_sb[(c, j)] = qt

                def v_sweep(vg, j=j, vlo=vlo, w=w):
                    pv = [pp_proj.tile([P, DQ], F32, tag="ps", name=f"pv{j}_{vg}_{_i}")
                          for _i in range(2)]
                    for kh2 in range(NKH2):
                        for mm in range(2):
                            m = 2 * vg + mm
                            nc.tensor.matmul(
                                pv[mm][:, 0:w],
                                ht_sb[(j, kh2)][:, :, P * m:P * m + P],
                                wv_sb[kh2][:, :, vlo:DQ],
                                start=(kh2 == 0), stop=False,
                                perf_mode=mybir.MatmulPerfMode.DoubleRow,
                            )
                    for mm in range(2):
                        m = 4 * j + 2 * vg + mm
                        nc.tensor.matmul(
                            pv[mm][:, 0:w], ones16[:], bv_sb[:, vlo:DQ],
                            start=False, stop=True)
                        if m % 2 == 0:
                            vt = v_pool.tile([P, 2, DQ], F8, tag="v",
                                             name=f"v{m // 2}")
                            v_sb[m // 2] = vt
                        nc.scalar.activation(
                            v_sb[m // 2][:, m % 2, vlo:DQ], pv[mm][:, 0:w],
                            mybir.ActivationFunctionType.Identity,
                            scale=1.0 / SW)

                for t in range(0, len(cols), 2):
                    sweeps.append(lambda cp=tuple(cols[t:t + 2]): qk_sweep(cp))
                for vg in range(2):
                    sweeps.append(lambda vg=vg: v_sweep(vg))
                return sweeps

            def needed_tiles(h, J):
                w = set(wins[h])
                nd = [i for i in range(NKT) if tilemap[(i, J)] != 'skip' and i in w]
                assert nd == list(range(len(nd))), "window must be a prefix"
                return nd

            def attn_head(h, J, pctx, pd):
                """Emit one pair-step generator for head h, block J."""
                needed = needed_tiles(h, J)
                npair = (len(needed) + 1) // 2
                for pr in range(npair):
                    pt2 = prob_pool.tile([P, 2, 512], F8, tag="pt")
                    halves = needed[2 * pr:2 * pr + 2]
                    for half, i in enumerate(halves):
                        pscore = pp_score.tile([P, 512], F32, tag="pscore")
                        nc.tensor.matmul(
                            pscore[:],
                            qkt_sb[(HPC + h, i // 4)][:, P * (i % 4):P * (i % 4) + P],
                            qkt_sb[(h, J)][:],
                            start=True, stop=True,
                        )
                        pat = tilemap[(i, J)]
                        if pat is not None:
                            nc.vector.tensor_add(
                                pscore[:], pscore[:],
                                mask_sb[:, 512 * pat:512 * pat + 512])
                        nc.scalar.activation(
                            pt2[:, half, :], pscore[:],
                            mybir.ActivationFunctionType.Exp,
                            bias=al_sb[:, h * NKT + i:h * NKT + i + 1],
                        )
                    if len(halves) == 1:
                        nc.any.memset(pt2[:, 1, :], 0.0)
                    yield
                    nc.tensor.matmul(
                        pctx[:],
                        v_sb[needed[2 * pr] // 2][:, :, P * h:P * h + P],
                        pt2[:],
                        start=(pr == 0), stop=(pr == npair - 1),
                        perf_mode=mybir.MatmulPerfMode.DoubleRow,
                    )
                    nc.tensor.matmul(
                        pd[:], ones8[:], pt2[:],
                        start=(pr == 0), stop=(pr == npair - 1),
                        perf_mode=mybir.MatmulPerfMode.DoubleRow,
                    )
                    yield

            def finish_head(h, J, pctx, pd):
                deps = dstat_pool.tile([1, 512], F32, tag="deps")
                nc.vector.tensor_scalar_add(deps[:], pd[0:1, :], 1e-12)
                rec = dstat_pool.tile([1, 512], F32, tag="rec")
                nc.vector.reciprocal_approx_fast(rec[:], deps[:])
                recb = dstat_pool.tile([P, 512], F32, tag="recb")
                nc.gpsimd.partition_broadcast(recb[:], rec[:], 128)
                ct = ctxt_pool.tile([P, 512], BF16, tag="ctxt")
                nc.vector.tensor_mul(ct[:], pctx[:], recb[:])
                ctxt_sb[(h, J)] = ct

            def attn_block(J):
                """Generator: yields after each interleaved 2-head round."""
                for hp in range(2):
                    h0, h1 = 2 * hp, 2 * hp + 1
                    pctx0 = pp_ctx.tile([P, 512], F32, tag="pctx", name=f"pc{J}_{h0}")
                    pctx1 = pp_ctx.tile([P, 512], F32, tag="pctx", name=f"pc{J}_{h1}")
                    pd0 = pp_d.tile([32, 512], F32, tag="pd", name=f"pd{J}_{h0}")
                    pd1 = pp_d.tile([32, 512], F32, tag="pd", name=f"pd{J}_{h1}")
                    gens = [(attn_head(h0, J, pctx0, pd0), h0, pctx0, pd0),
                            (attn_head(h1, J, pctx1, pd1), h1, pctx1, pd1)]
                    live = list(gens)
                    while live:
                        for item in list(live):
                            g, h, pc, pdd = item
                            try:
                                next(g)
                            except StopIteration:
                                finish_head(h, J, pc, pdd)
                                live.remove(item)
                        yield

            def dense_groups(J, pools=None):
                groups = []
                pools = pools or [pp_proj]
                obig = {}

                def grp(cb, mm, J=J):
                    pool = pools[(4 * mm + cb) % len(pools)]
                    tg = "ps" if pool is pp_proj else "pscore"
                    m = 4 * J + mm
                    pdn = pool.tile([P, 512], F32, tag=tg, name=f"dn{J}_{cb}_{mm}")
                    for kd in range(HPC):
                        nc.tensor.matmul(
                            pdn[:],
                            ctxt_sb[(kd, J)][:, P * mm:P * mm + P],
                            wd_sb[(kd, cb)][:],
                            start=(kd == 0), stop=(kd == HPC - 1),
                        )
                    if mm not in obig:
                        obig[mm] = ostage_pool.tile(
                            [P, H], BF16, tag="ot", name=f"ob{J}_{mm}")
                    nc.vector.tensor_scalar_mul(
                        obig[mm][:, 512 * cb:512 * cb + 512], pdn[:], 1.0 / SV)
                    if cb == NJ - 1:
                        nc.sync.dma_start(
                            out=out[P * m:P * m + P, :], in_=obig[mm][:])

                for mm in range(4):
                    for cb in range(NJ):
                        groups.append(lambda cb=cb, mm=mm: grp(cb, mm))
                return groups

            # ---- master schedule: proj(j) sweeps interleaved with
            # attn(j-1) rounds; dense groups pumped into attn(3) bubbles ----
            dense_q = []

            def pump(n):
                while n > 0 and dense_q:
                    dense_q.pop(0)()
                    n -= 1

            for sw in proj_sweeps(0):
                sw()
            for j in range(1, NJ):
                rounds = attn_block(j - 1)
                sweeps = proj_sweeps(j)
                nround = 0
                for ha, hb in ((0, 1), (2, 3)):
                    npa = (len(needed_tiles(ha, j - 1)) + 1) // 2
                    npb = (len(needed_tiles(hb, j - 1)) + 1) // 2
                    nround += 2 * max(npa, npb) + 2
                per = max(1, (nround + len(sweeps) - 1) // len(sweeps))
                exhausted = False
                for sw in sweeps:
                    sw()
                    for _ in range(per):
                        try:
                            next(rounds)
                        except StopIteration:
                            exhausted = True
                            break
                    pump(1)
                while not exhausted:
                    try:
                        next(rounds)
                        pump(1)
                    except StopIteration:
                        exhausted = True
                dense_q.extend(dense_groups(j - 1))
            for _ in attn_block(NJ - 1):
                pump(3)
            dense_q.extend(dense_groups(NJ - 1, pools=[pp_proj, pp_score]))
            pump(len(dense_q))

    nc.finalize()
    return nc


def _pack_dr(mat, scale):
    """[H, C] f32 -> [128, NKH2, 2, C] fp8 DoubleRow pairs, scaled."""
    h, c = mat.shape
    m = np.clip(mat * scale, -448.0, 448.0).astype(E4)
    return np.ascontiguousarray(m.reshape(NKH2, 2, P, c).transpose(2, 0, 1, 3))


def kernel(hidden_states, attention_mask, residual, alibi, Wqkv, bqkv, Wd, bd):
    hidden_states = np.asarray(hidden_states, np.float32)
    attention_mask = np.asarray(attention_mask).astype(bool)
    residual = np.asarray(residual, np.float32)
    alibi = np.asarray(alibi, np.float32)
    Wqkv = np.asarray(Wqkv, np.float32)
    bqkv = np.asarray(bqkv, np.float32)
    Wd = np.asarray(Wd, np.float32)
    bd = np.asarray(bd, np.float32)

    m0 = attention_mask[0, 0]
    for b in range(1, B):
        assert np.array_equal(attention_mask[b, 0], m0), "per-batch masks differ"
    tilemap, patterns = _analyze_mask(m0)
    npat = len(patterns)
    assert npat <= 8, f"too many mask patterns: {npat}"
    mask_host = np.ascontiguousarray(
        np.concatenate(patterns, axis=1) if npat else np.zeros((P, 512), np.float32))

    # per-head alibi windows: keep k-tile i iff max alibi in tile >= -THR.
    # Skipped tiles have unnormalized probs <= e^(s_max - THR) ~ e^-22 -> 0.
    THR = 28.0
    keep = []
    for h in range(NH):
        km = tuple(
            bool(max(np.max(alibi[b * NH + h, 0, P * i:P * i + P]) for b in range(B))
                 >= -THR)
            for i in range(NKT))
        keep.append(km)
    order = sorted(range(NH), key=lambda h: sum(keep[h]))
    slots = [order[HPC * t:HPC * t + HPC] for t in range(HPC)]
    wins = tuple(
        tuple(i for i in range(NKT) if any(keep[h][i] for h in sl))
        for sl in slots)

    key = tuple(sorted((k, str(v)) for k, v in tilemap.items())) + (npat, wins)
    if key not in _cache:
        _cache[key] = _build_program(tilemap, npat, wins)
    nc = _cache[key]

    Wq3 = Wqkv.reshape(H, NH, 3, HD)   # col = nh*384 + {0:k,1:q,2:v}*128 + d
    bq3 = bqkv.reshape(NH, 3, HD)

    hT_cores = {}
    for b in range(B):
        p = _pack_dr(np.ascontiguousarray(hidden_states[b].T), SH)  # [P,NKH2,2,S]
        hT_cores[b] = np.ascontiguousarray(
            p.reshape(P, NKH2, 2, NJ, 512).transpose(0, 3, 1, 2, 4))

    in_maps = []
    for core in range(NCORES):
        b, g = divmod(core, HPC)
        hs = [slots[t][g] for t in range(HPC)]
        Wq = np.concatenate([Wq3[:, h, 1, :] for h in hs], 1)
        Wk = np.concatenate([Wq3[:, h, 0, :] for h in hs], 1)
        Wv_ = np.concatenate([Wq3[:, h, 2, :] for h in hs], 1)
        bq = np.concatenate([bq3[h, 1, :] for h in hs]) * ALPHA
        bk = np.concatenate([bq3[h, 0, :] for h in hs])
        bv_ = np.concatenate([bq3[h, 2, :] for h in hs])
        bqk_h = np.concatenate([bq, bk]).reshape(NCOL, P).T
        al_h = np.stack([alibi[b * NH + h, 0] for h in hs], 0) + LN_SP  # [HPC, S]
        al_sb = al_h.reshape(HPC, NKT, P).transpose(2, 0, 1).reshape(P, HPC * NKT)
        in_maps.append({
            "hT": hT_cores[b],
            "Wqk": np.ascontiguousarray(np.stack(
                [_pack_dr(Wq, SW), _pack_dr(Wk, SW)], axis=1)),
            "Wv": _pack_dr(Wv_, SW),
            "bqk": np.ascontiguousarray(bqk_h, dtype=np.float32),
            "bv": np.clip(bv_ * 2048.0, -448.0, 448.0).reshape(1, DQ).astype(E4),
            "alib": np.ascontiguousarray(al_sb, dtype=np.float32),
            "Wd": np.ascontiguousarray(
                np.stack([Wd[h * HD:(h + 1) * HD, :] for h in hs], 0)
                .transpose(1, 0, 2)).astype(BF),
            "masks": mask_host,
        })

    res = run_bass_kernel_spmd(nc, in_maps, list(range(NCORES)))

    outp = np.zeros((B, S, H), np.float32)
    for core in range(NCORES):
        outp[core // HPC] += res.results[core]["out"].astype(np.float32)
    outp += bd[None, None, :] + residual
    return outp


# revision 20
# speedup vs baseline: 1.3423x; 1.0093x over previous
"""BLOOM-style attention block (QKV proj + ALiBi causal attention + dense + residual)
for Trainium2, SPMD over 8 NeuronCores.

Sharding: core c -> (b = c // 4, head group g = c % 4, heads [4g..4g+4)).

v2: fp8 (e4m3) DoubleRow matmuls for the QKV projections and probs@V;
bf16 for scores and dense (error budget: ~1.6e-2 vs 2e-2 gate).

Per core (b, g):
  - hT packed as K=256 DoubleRow pairs [128, 8, 2, S] fp8 (x32 scale)
  - q/k proj DR (Wqk fp8 x1024): 8 DR matmuls per [128,512] tile; ACT applies
    scale (alpha/32768 for q, 1/32768 for k) + bias -> bf16 tiles
  - v proj DR -> +bias via ones(16)xfp8(bv*2048) matmul -> ACT x(1/1024) -> fp8 (32*v)
    stored as k-tile pairs v2 [128, 2, DQ]
  - scores bf16 transposed per (head, q-block): s_T [k128, q512]
  - additive -30000 mask (DVE) on partial tiles; exp via ACT with per-partition
    alibi+ln2 bias -> fp8 probs (2*p) written into pair tiles pt2 [128, 2, 512]
  - ctx DR (v2 x pt2) accumulated in PSUM; denominator via ones-DR matmul into
    a [1,512] PSUM accumulator (no DVE/gpsimd reduction)
  - rec = 1/d (DVE), broadcast via gpsimd, ctx normalized by DVE mul -> bf16
  - dense bf16 per q-block interleaved after each attn block; ACT copy x(1/32)
Host: out[b] = sum_g partial[b,g] + bd + residual[b].
"""

import numpy as np
import ml_dtypes

import concourse.bacc as bacc
import concourse.mybir as mybir
from concourse import tile
from concourse.bass_utils import run_bass_kernel_spmd

B, S, H, NH, HD = 2, 2048, 2048, 16, 128
ALPHA = 1.0 / float(np.sqrt(HD))
P = 128
NCORES = 8
HPC = 4            # heads per core
DQ = HPC * HD      # 512 = per-core q/k/v width
NJ = S // 512      # 4 q blocks of 512
NKT = S // P       # 16 k tiles of 128
NKH2 = H // 256    # 8 DoubleRow contraction pairs for projections
NCOL = 2 * DQ // P  # 8 qk col tiles (c<4: q head c, c>=4: k head c-4)
F32 = mybir.dt.float32
BF16 = mybir.dt.bfloat16
F8 = mybir.dt.float8e4
BF = ml_dtypes.bfloat16
E4 = ml_dtypes.float8_e4m3fn

SH = 32.0        # hidden fp8 scale
SW = 1024.0      # weight fp8 scale
SV = 32.0        # v fp8 scale (= SH*SW / 1024)
SP = 1.0         # prob fp8 scale (folded into alibi bias; 1.0: max unnorm prob ~167 << 448)
LN_SP = float(np.log(SP))

_cache = {}


def _analyze_mask(mask_b):
    """mask_b: [S, S] bool, True == masked out. Per (i, J) k/q tile:
    'skip' | pattern-index | None. Patterns are additive [k128, q512] f32."""
    patterns, pat_index, tilemap = [], {}, {}
    for J in range(NJ):
        for i in range(NKT):
            sub = mask_b[512 * J:512 * J + 512, P * i:P * i + P]  # [q, k]
            if sub.all():
                tilemap[(i, J)] = 'skip'
            elif not sub.any():
                tilemap[(i, J)] = None
            else:
                pat = np.where(sub.T, np.float32(-30000.0), np.float32(0.0))
                key = pat.tobytes()
                if key not in pat_index:
                    pat_index[key] = len(patterns)
                    patterns.append(pat)
                tilemap[(i, J)] = pat_index[key]
    return tilemap, patterns


def _build_program(tilemap, npat, wins):
    # wins: per head-slot, tuple of kept k-tile indices (alibi window)
    nc = bacc.Bacc(None, target_bir_lowering=False, debug=False)

    hT = nc.dram_tensor("hT", [P, NJ, NKH2, 2, 512], F8, kind="ExternalInput")
    Wqk = nc.dram_tensor("Wqk", [P, 2, NKH2, 2, DQ], F8, kind="ExternalInput")
    Wv = nc.dram_tensor("Wv", [P, NKH2, 2, DQ], F8, kind="ExternalInput")
    bqk = nc.dram_tensor("bqk", [P, NCOL], F32, kind="ExternalInput")
    bv = nc.dram_tensor("bv", [1, DQ], F8, kind="ExternalInput")
    alib = nc.dram_tensor("alib", [P, HPC * NKT], F32, kind="ExternalInput")
    Wd = nc.dram_tensor("Wd", [P, HPC, H], BF16, kind="ExternalInput")
    masks = nc.dram_tensor("masks", [P, max(npat, 1) * 512], F32, kind="ExternalInput")
    out = nc.dram_tensor("out", [S, H], BF16, kind="ExternalOutput")

    with tile.TileContext(nc) as tc:
        with (
            tc.tile_pool(name="wqk", bufs=4) as wqk_pool,
            tc.tile_pool(name="wv", bufs=1) as wv_pool,
            tc.tile_pool(name="wd", bufs=1) as wd_pool,
            tc.tile_pool(name="consts", bufs=1) as consts,
            tc.tile_pool(name="hts", bufs=NJ - 1) as ht_pool,
            tc.tile_pool(name="qkt", bufs=NCOL * NJ + 1) as qkt_pool,
            tc.tile_pool(name="vsb", bufs=NKT // 2 + 1) as v_pool,
            tc.tile_pool(name="prob", bufs=5) as prob_pool,
            tc.tile_pool(name="dstat", bufs=7) as dstat_pool,
            tc.tile_pool(name="ctxt", bufs=HPC * NJ + 1) as ctxt_pool,
            tc.tile_pool(name="ostage", bufs=4) as ostage_pool,
            tc.tile_pool(name="pp_proj", bufs=2, space="PSUM") as pp_proj,
            tc.tile_pool(name="pp_score", bufs=2, space="PSUM") as pp_score,
            tc.tile_pool(name="pp_ctx", bufs=2, space="PSUM") as pp_ctx,
            tc.tile_pool(name="pp_d", bufs=2, space="PSUM") as pp_d,
        ):
            # ---- constants ----
            mask_sb = consts.tile([P, max(npat, 1) * 512], F32)
            nc.sync.dma_start(out=mask_sb[:], in_=masks[:])
            al_sb = consts.tile([P, HPC * NKT], F32)
            nc.sync.dma_start(out=al_sb[:], in_=alib[:])
            bqk_sb = consts.tile([P, NCOL], F32)
            nc.sync.dma_start(out=bqk_sb[:], in_=bqk[:])
            bv_sb = consts.tile([1, DQ], F8)
            nc.sync.dma_start(out=bv_sb[:], in_=bv[:])
            ones16 = consts.tile([1, P], F8)
            nc.any.memset(ones16[:], 16.0)
            ones8 = consts.tile([P, 2, 32], F8)
            nc.any.memset(ones8[:], 1.0)

            # ---- batched input DMAs (issue cost ~0.66us each on the Sync
            # queue dominates with many small transfers): one DMA per hT
            # block, two for Wqk (lo/hi cols), one each for Wv / Wd ----
            ht_sb = {}  # (j, kh2) -> [128, 2, 512] fp8 view

            def load_ht(j):
                t = ht_pool.tile([P, NKH2, 2, 512], F8, tag="ht", name=f"htb{j}")
                nc.sync.dma_start(out=t[:], in_=hT[:, j, :, :, :])
                for kh2 in range(NKH2):
                    ht_sb[(j, kh2)] = t[:, kh2, :, :]

            hh = NKH2 // 2
            # one tile per DMA: dependency tracking is per-tile, so sharing a
            # tile across DMAs makes early readers wait for ALL its writers
            ht0_t = []
            wqk_t = [[None, None], [None, None]]  # [half][kh2 group]
            for grp in range(2):
                t = consts.tile([P, hh, 2, 512], F8, tag=f"ht0_{grp}", name=f"htb0_{grp}")
                nc.sync.dma_start(
                    out=t[:], in_=hT[:, 0, hh * grp:hh * grp + hh, :, :])
                ht0_t.append(t)
                w = wqk_pool.tile([P, hh, 2, DQ], F8, tag="wqk", name=f"wqlo{grp}")
                nc.sync.dma_start(
                    out=w[:], in_=Wqk[:, 0, hh * grp:hh * grp + hh, :, :])
                wqk_t[0][grp] = w
            for kh2 in range(NKH2):
                ht_sb[(0, kh2)] = ht0_t[kh2 // hh][:, kh2 % hh, :, :]
            for grp in range(2):
                w = wqk_pool.tile([P, hh, 2, DQ], F8, tag="wqk", name=f"wqhi{grp}")
                nc.sync.dma_start(
                    out=w[:], in_=Wqk[:, 1, hh * grp:hh * grp + hh, :, :])
                wqk_t[1][grp] = w

            def wqk_view(kh2, c):
                # c<4: q head c in the lo half; c>=4: k head c-4 in the hi half
                h, cc = (0, c) if c < HPC else (1, c - HPC)
                return wqk_t[h][kh2 // hh][:, kh2 % hh, :, P * cc:P * cc + P]
            wv_big = wv_pool.tile([P, NKH2, 2, DQ], F8, tag="wv")
            nc.sync.dma_start(out=wv_big[:], in_=Wv[:])
            wv_sb = [wv_big[:, kh2, :, :] for kh2 in range(NKH2)]
            load_ht(1)
            load_ht(2)
            load_ht(3)
            wd_big = wd_pool.tile([P, HPC, H], BF16, tag="wd")
            nc.sync.dma_start(out=wd_big[:], in_=Wd[:])
            wd_sb = {}  # (kd, cb) -> [128, 512] bf16 view
            for kd in range(HPC):
                for cb in range(NJ):
                    wd_sb[(kd, cb)] = wd_big[:, kd, 512 * cb:512 * cb + 512]

            qkt_sb = {}   # (c, j) -> [128, 512] bf16; c<4: q head c (alpha-scaled), c>=4: k
            v_sb = {}     # pr -> [128, 2, DQ] fp8 (32*v), k-tiles (2pr, 2pr+1)
            ctxt_sb = {}  # (h, J) -> [128, 512] bf16 (32*ctx)

            def kslots(j):
                # head-slots whose alibi window reaches key block j
                return [t for t in range(HPC) if wins[t] and wins[t][-1] >= 4 * j]

            def proj_sweeps(j):
                sweeps = []
                cols = list(range(HPC)) + [HPC + t for t in kslots(j)]
                vlo = min(kslots(j)) * P
                w = DQ - vlo

                def qk_sweep(cpair, j=j):
                    ps = [pp_proj.tile([P, 512], F32, tag="ps",
                                       name=f"ps{j}_{c}") for c in cpair]
                    for kh2 in range(NKH2):
                        for cc, c in enumerate(cpair):
                            nc.tensor.matmul(
                                ps[cc][:],
                                wqk_view(kh2, c),
                                ht_sb[(j, kh2)][:],
                                start=(kh2 == 0), stop=(kh2 == NKH2 - 1),
                                perf_mode=mybir.MatmulPerfMode.DoubleRow,
                            )
                    for cc, c in enumerate(cpair):
                        sc = (ALPHA / (SH * SW)) if c < HPC else (1.0 / (SH * SW))
                        qt = qkt_pool.tile([P, 512], BF16, tag="qkt")
                        nc.vector.tensor_scalar(
                            qt[:], ps[cc][:], sc, bqk_sb[:, c:c + 1],
                            mybir.AluOpType.mult, mybir.AluOpType.add)
                        qkt_sb[(c, j)] = qt

                def v_sweep(vg, j=j, vlo=vlo, w=w):
                    pv = [pp_proj.tile([P, DQ], F32, tag="ps", name=f"pv{j}_{vg}_{_i}")
                          for _i in range(2)]
                    for kh2 in range(NKH2):
                        for mm in range(2):
                            m = 2 * vg + mm
                            nc.tensor.matmul(
                                pv[mm][:, 0:w],
                                ht_sb[(j, kh2)][:, :, P * m:P * m + P],
                                wv_sb[kh2][:, :, vlo:DQ],
                                start=(kh2 == 0), stop=False,
                                perf_mode=mybir.MatmulPerfMode.DoubleRow,
                            )
                    for mm in range(2):
                        m = 4 * j + 2 * vg + mm
                        nc.tensor.matmul(
                            pv[mm][:, 0:w], ones16[:], bv_sb[:, vlo:DQ],
                            start=False, stop=True)
                        if m % 2 == 0:
                            vt = v_pool.tile([P, 2, DQ], F8, tag="v",
                                             name=f"v{m // 2}")
                            v_sb[m // 2] = vt
                        nc.scalar.activation(
                            v_sb[m // 2][:, m % 2, vlo:DQ], pv[mm][:, 0:w],
                            mybir.ActivationFunctionType.Identity,
                            scale=1.0 / SW)

                for t in range(0, len(cols), 2):
                    sweeps.append(lambda cp=tuple(cols[t:t + 2]): qk_sweep(cp))
                for vg in range(2):
                    sweeps.append(lambda vg=vg: v_sweep(vg))
                return sweeps

            def needed_tiles(h, J):
                w = set(wins[h])
                nd = [i for i in range(NKT) if tilemap[(i, J)] != 'skip' and i in w]
                assert nd == list(range(len(nd))), "window must be a prefix"
                return nd

            def attn_head(h, J, pctx, pd):
                """Emit one pair-step generator for head h, block J."""
                needed = needed_tiles(h, J)
                npair = (len(needed) + 1) // 2
                for pr in range(npair):
                    pt2 = prob_pool.tile([P, 2, 512], F8, tag="pt")
                    halves = needed[2 * pr:2 * pr + 2]
                    for half, i in enumerate(halves):
                        pscore = pp_score.tile([P, 512], F32, tag="pscore")
                        nc.tensor.matmul(
                            pscore[:],
                            qkt_sb[(HPC + h, i // 4)][:, P * (i % 4):P * (i % 4) + P],
                            qkt_sb[(h, J)][:],
                            start=True, stop=True,
                        )
                        pat = tilemap[(i, J)]
                        if pat is not None:
                            nc.vector.tensor_add(
                                pscore[:], pscore[:],
                                mask_sb[:, 512 * pat:512 * pat + 512])
                        nc.scalar.activation(
                            pt2[:, half, :], pscore[:],
                            mybir.ActivationFunctionType.Exp,
                            bias=al_sb[:, h * NKT + i:h * NKT + i + 1],
                        )
                    if len(halves) == 1:
                        nc.any.memset(pt2[:, 1, :], 0.0)
                    yield
                    nc.tensor.matmul(
                        pctx[:],
                        v_sb[needed[2 * pr] // 2][:, :, P * h:P * h + P],
                        pt2[:],
                        start=(pr == 0), stop=(pr == npair - 1),
                        perf_mode=mybir.MatmulPerfMode.DoubleRow,
                    )
                    nc.tensor.matmul(
                        pd[:], ones8[:], pt2[:],
                        start=(pr == 0), stop=(pr == npair - 1),
                        perf_mode=mybir.MatmulPerfMode.DoubleRow,
                    )
                    yield

            def finish_head(h, J, pctx, pd):
                deps = dstat_pool.tile([1, 512], F32, tag="deps")
                nc.vector.tensor_scalar_add(deps[:], pd[0:1, :], 1e-12)
                rec = dstat_pool.tile([1, 512], F32, tag="rec")
                nc.vector.reciprocal_approx_fast(rec[:], deps[:])
                recb = dstat_pool.tile([P, 512], F32, tag="recb")
                nc.gpsimd.partition_broadcast(recb[:], rec[:], 128)
                ct = ctxt_pool.tile([P, 512], BF16, tag="ctxt")
                nc.vector.tensor_mul(ct[:], pctx[:], recb[:])
                ctxt_sb[(h, J)] = ct

            def attn_block(J):
                """Generator: yields after each interleaved 2-head round."""
                for hp in range(2):
                    h0, h1 = 2 * hp, 2 * hp + 1
                    pctx0 = pp_ctx.tile([P, 512], F32, tag="pctx", name=f"pc{J}_{h0}")
                    pctx1 = pp_ctx.tile([P, 512], F32, tag="pctx", name=f"pc{J}_{h1}")
                    pd0 = pp_d.tile([32, 512], F32, tag="pd", name=f"pd{J}_{h0}")
                    pd1 = pp_d.tile([32, 512], F32, tag="pd", name=f"pd{J}_{h1}")
                    gens = [(attn_head(h0, J, pctx0, pd0), h0, pctx0, pd0),
                            (attn_head(h1, J, pctx1, pd1), h1, pctx1, pd1)]
                    live = list(gens)
                    while live:
                        for item in list(live):
                            g, h, pc, pdd = item
                            try:
                                next(g)
                            except StopIteration:
                                finish_head(h, J, pc, pdd)
                                live.remove(item)
                        yield

            def dense_groups(J, pools=None):
                groups = []
                pools = pools or [pp_proj]
                obig = {}

                def grp(cb, mm, J=J):
                    pool = pools[(4 * mm + cb) % len(pools)]
                    tg = "ps" if pool is pp_proj else "pscore"
                    m = 4 * J + mm
                    pdn = pool.tile([P, 512], F32, tag=tg, name=f"dn{J}_{cb}_{mm}")
                    for kd in range(HPC):
                        nc.tensor.matmul(
                            pdn[:],
                            ctxt_sb[(kd, J)][:, P * mm:P * mm + P],
                            wd_sb[(kd, cb)][:],
                            start=(kd == 0), stop=(kd == HPC - 1),
                        )
                    half = cb // 2
                    key = (mm, half)
                    if key not in obig:
                        obig[key] = ostage_pool.tile(
                            [P, H // 2], BF16, tag="ot", name=f"ob{J}_{mm}_{half}")
                    nc.vector.tensor_scalar_mul(
                        obig[key][:, 512 * (cb % 2):512 * (cb % 2) + 512],
                        pdn[:], 1.0 / SV)
                    if cb % 2 == 1:
                        nc.sync.dma_start(
                            out=out[P * m:P * m + P,
                                    1024 * half:1024 * half + 1024],
                            in_=obig[key][:])

                for mm in range(4):
                    for cb in range(NJ):
                        groups.append(lambda cb=cb, mm=mm: grp(cb, mm))
                return groups

            # ---- master schedule: proj(j) sweeps interleaved with
            # attn(j-1) rounds; dense groups pumped into attn(3) bubbles ----
            dense_q = []

            def pump(n):
                while n > 0 and dense_q:
                    dense_q.pop(0)()
                    n -= 1

            for sw in proj_sweeps(0):
                sw()
            for j in range(1, NJ):
                rounds = attn_block(j - 1)
                sweeps = proj_sweeps(j)
                nround = 0
                for ha, hb in ((0, 1), (2, 3)):
                    npa = (len(needed_tiles(ha, j - 1)) + 1) // 2
                    npb = (len(needed_tiles(hb, j - 1)) + 1) // 2
                    nround += 2 * max(npa, npb) + 2
                per = max(1, (nround + len(sweeps) - 1) // len(sweeps))
                exhausted = False
                for sw in sweeps:
                    sw()
                    for _ in range(per):
                        try:
                            next(rounds)
                        except StopIteration:
                            exhausted = True
                            break
                    pump(1)
                while not exhausted:
                    try:
                        next(rounds)
                        pump(1)
                    except StopIteration:
                        exhausted = True
                dense_q.extend(dense_groups(j - 1))
            for _ in attn_block(NJ - 1):
                pump(3)
            dense_q.extend(dense_groups(NJ - 1, pools=[pp_proj, pp_score]))
            pump(len(dense_q))

    nc.finalize()
    return nc


def _pack_dr(mat, scale):
    """[H, C] f32 -> [128, NKH2, 2, C] fp8 DoubleRow pairs, scaled."""
    h, c = mat.shape
    m = np.clip(mat * scale, -448.0, 448.0).astype(E4)
    return np.ascontiguousarray(m.reshape(NKH2, 2, P, c).transpose(2, 0, 1, 3))


def kernel(hidden_states, attention_mask, residual, alibi, Wqkv, bqkv, Wd, bd):
    hidden_states = np.asarray(hidden_states, np.float32)
    attention_mask = np.asarray(attention_mask).astype(bool)
    residual = np.asarray(residual, np.float32)
    alibi = np.asarray(alibi, np.float32)
    Wqkv = np.asarray(Wqkv, np.float32)
    bqkv = np.asarray(bqkv, np.float32)
    Wd = np.asarray(Wd, np.float32)
    bd = np.asarray(bd, np.float32)

    m0 = attention_mask[0, 0]
    for b in range(1, B):
        assert np.array_equal(attention_mask[b, 0], m0), "per-batch masks differ"
    tilemap, patterns = _analyze_mask(m0)
    npat = len(patterns)
    assert npat <= 8, f"too many mask patterns: {npat}"
    mask_host = np.ascontiguousarray(
        np.concatenate(patterns, axis=1) if npat else np.zeros((P, 512), np.float32))

    # per-head alibi windows: keep k-tile i iff max alibi in tile >= -THR.
    # Skipped tiles have unnormalized probs <= e^(s_max - THR) ~ e^-22 -> 0.
    THR = 28.0
    keep = []
    for h in range(NH):
        km = tuple(
            bool(max(np.max(alibi[b * NH + h, 0, P * i:P * i + P]) for b in range(B))
                 >= -THR)
            for i in range(NKT))
        keep.append(km)
    order = sorted(range(NH), key=lambda h: sum(keep[h]))
    slots = [order[HPC * t:HPC * t + HPC] for t in range(HPC)]
    wins = tuple(
        tuple(i for i in range(NKT) if any(keep[h][i] for h in sl))
        for sl in slots)

    key = tuple(sorted((k, str(v)) for k, v in tilemap.items())) + (npat, wins)
    if key not in _cache:
        _cache[key] = _build_program(tilemap, npat, wins)
    nc = _cache[key]

    Wq3 = Wqkv.reshape(H, NH, 3, HD)   # col = nh*384 + {0:k,1:q,2:v}*128 + d
    bq3 = bqkv.reshape(NH, 3, HD)

    hT_cores = {}
    for b in range(B):
        p = _pack_dr(np.ascontiguousarray(hidden_states[b].T), SH)  # [P,NKH2,2,S]
        hT_cores[b] = np.ascontiguousarray(
            p.reshape(P, NKH2, 2, NJ, 512).transpose(0, 3, 1, 2, 4))

    in_maps = []
    for core in range(NCORES):
        b, g = divmod(core, HPC)
        hs = [slots[t][g] for t in range(HPC)]
        Wq = np.concatenate([Wq3[:, h, 1, :] for h in hs], 1)
        Wk = np.concatenate([Wq3[:, h, 0, :] for h in hs], 1)
        Wv_ = np.concatenate([Wq3[:, h, 2, :] for h in hs], 1)
        bq = np.concatenate([bq3[h, 1, :] for h in hs]) * ALPHA
        bk = np.concatenate([bq3[h, 0, :] for h in hs])
        bv_ = np.concatenate([bq3[h, 2, :] for h in hs])
        bqk_h = np.concatenate([bq, bk]).reshape(NCOL, P).T
        al_h = np.stack([alibi[b * NH + h, 0] for h in hs], 0) + LN_SP  # [HPC, S]
        al_sb = al_h.reshape(HPC, NKT, P).transpose(2, 0, 1).reshape(P, HPC * NKT)
        in_maps.append({
            "hT": hT_cores[b],
            "Wqk": np.ascontiguousarray(np.stack(
                [_pack_dr(Wq, SW), _pack_dr(Wk, SW)], axis=1)),
            "Wv": _pack_dr(Wv_, SW),
            "bqk": np.ascontiguousarray(bqk_h, dtype=np.float32),
            "bv": np.clip(bv_ * 2048.0, -448.0, 448.0).reshape(1, DQ).astype(E4),
            "alib": np.ascontiguousarray(al_sb, dtype=np.float32),
            "Wd": np.ascontiguousarray(
                np.stack([Wd[h * HD:(h + 1) * HD, :] for h in hs], 0)
                .transpose(1, 0, 2)).astype(BF),
            "masks": mask_host,
        })

    res = run_bass_kernel_spmd(nc, in_maps, list(range(NCORES)))

    outp = np.zeros((B, S, H), np.float32)
    for core in range(NCORES):
        outp[core // HPC] += res.results[core]["out"].astype(np.float32)
    outp += bd[None, None, :] + residual
    return outp
